# revision 24
# baseline (speedup 1.0000x reference)
"""Trainium2 Bass kernel for nn_Device_Policy (segment_reduce).

Strategy: shard the node axis N across 8 NeuronCores.  Host-side the
inputs are repacked into DMA/PE-friendly layouts and compressed:

  - mpnn_forward -> bf16, packed [128, 32768] so partition p holds node
    b*128+p of the shard for block b (free offset b*128+h).
  - state -> bf16, packed [128, 16384]: features on partitions, the two
    node-halves of the shard stacked (partition f = feat f of nodes
    0..16383, partition 64+f = feat f of nodes 16384..32767).  Column
    sums / sums-of-squares then reduce along the free axis.
  - mask -> fp8 e4m3 bytes (0x00/0x38), pre-transposed and packed
    [128, 16384] with partition p = node b*128+p, free b*64+d.

Per core the masked segment sum dse[d, h1] accumulates in one PSUM bank
over 256 matmuls (lhsT = fp8 mask block [128n, 64d] stationary, rhs =
bf16 mpnn block [128n, 128h] moving).  The [128, 66] pack (dse.T +
state sums) is exchanged via raw peer-to-peer SWDGE remote DMA (XOR
slotting: core c sends its pack to peer c^k's slot k, k=1..7) instead
of the CC-library AllReduce (which costs ~35us for 34KB); each core
sums the 8 slots and runs the small replicated MLP head.
"""

import sys

if "/opt/trn_rl_repo" not in sys.path:
    sys.path.insert(0, "/opt/trn_rl_repo")

import numpy as np
import ml_dtypes

import concourse.bacc as bacc
import concourse.bass as bass
import concourse.mybir as mybir
import concourse.tile as tile
from concourse import masks
from concourse.bass_utils import run_bass_kernel_spmd

NCORES = 8
N = 262144
F = 64
D = 64
DF = 32
H1 = 128
H2 = 64
NSH = N // NCORES          # nodes per core = 32768
NBLK = NSH // 128          # 128-node matmul blocks per core = 256
NSUP = 8                   # DMA super-tiles
BPS = NBLK // NSUP         # blocks per super-tile = 32
MPW = NSH * H1 // 128      # mpnn pack free width = 32768
STW = NSH * F // 128       # state pack free width = 16384
MKW = NSH * D // 128       # mask pack free width = 16384
EPS = 1e-6
SLOPE = 0.1
PK = 66                    # pack columns: 64 dse.T + ssum + ssq

f32 = mybir.dt.float32
bf16 = mybir.dt.bfloat16
f8e4 = mybir.dt.float8e4
u8 = mybir.dt.uint8
ADD = mybir.AluOpType.add
MUL = mybir.AluOpType.mult
SUB = mybir.AluOpType.subtract
AX = mybir.AxisListType.X
IDENT = mybir.ActivationFunctionType.Identity
SQUARE = mybir.ActivationFunctionType.Square
SQRT = mybir.ActivationFunctionType.Sqrt

# cpack free-column layout (f32 columns in one packed const tensor)
C_DFST = 0      # [64, 64]   device_feat_state.T (padded 32->64 rows)
C_W1T = 64      # [64, 128]  W1.T padded
C_W2T = 192     # [64, 128]  W2.T
C_W3T = 320     # [128, 256] W3.T, 4 chunks of [128, 64]
C_B1 = 576      # [128, 1]
C_B2 = 577      # [128, 1]
C_B3 = 578      # [64, 1]
C_W4T = 579     # [64, 1]
C_B4 = 580      # [64, 1]
C_SPRED = 581   # [64, 1]
C_MPRED = 582   # [128, 1]
C_FOLD = 583    # [128, 64]  vertical-stacked I64: folds p and p+64
CW = 648


def build_program(stage="full"):
    nc = bacc.Bacc(
        "TRN2",
        target_bir_lowering=False,
        debug=False,
        enable_asserts=False,
        num_devices=NCORES,
        num_swdge_queues=4,
    )

    x_mpnn = nc.dram_tensor("x_mpnn", [128, MPW], bf16, kind="ExternalInput")
    x_state = nc.dram_tensor("x_state", [128, STW], bf16, kind="ExternalInput")
    x_maskT = nc.dram_tensor("x_maskT", [128, MKW], u8, kind="ExternalInput")
    x_cpack = nc.dram_tensor("x_cpack", [128, CW], f32, kind="ExternalInput")
    y_out = nc.dram_tensor("y_out", [D], f32, kind="ExternalOutput")

    hooks = {}
    with tile.TileContext(nc) as tc:
        emit(nc, tc, x_mpnn, x_state, x_maskT, x_cpack, y_out, hooks,
             stage=stage)

    # The arrival gate (rsem >= 14, incremented only by the 7 peers' remote
    # DMAs) is attached AFTER tile scheduling: the scheduling-pass CoreSim
    # has no peers, so an emit-time wait would deadlock it.  The gate is a
    # standalone DVE EventSemaphore (2 wait slots) emitted with a trivially
    # true rsem>=0 wait to pin its stream position; the real >=14 wait is
    # added here, before codegen.
    if stage == "full":
        import bass_rust
        gate_ins = hooks["gate"].ins
        bass_rust.wait_op(gate_ins, hooks["rsem"], 14, "sem-ge", True)

    nc.compile()

    if stage == "full":
        # Safety: the gate must precede the first remote-slot read (the
        # slot-sum add) in the final DVE stream, else the exchange races.
        seen_gate = False
        for f in nc.m.functions:
            for bb in f.blocks:
                for i in bb.instructions:
                    if i.name == gate_ins.name:
                        seen_gate = True
                    if i.name == hooks["first_read"].ins.name:
                        assert seen_gate, "gate scheduled after slot sums"
        assert seen_gate, "arrival gate missing from compiled program"
    return nc


def emit(nc, tc, x_mpnn, x_state, x_maskT, x_cpack, y_out, hooks,
         stage="full"):
    ctx_pools = []

    def pool(name, bufs, space="SBUF"):
        p = tc.tile_pool(name=name, bufs=bufs, space=space)
        ctx_pools.append(p)
        return p.__enter__()

    cpool = pool("const", 1)
    mpnn_pool = pool("mpnn", 5)
    mask_pool = pool("maskp", 4)
    state_pool = pool("state", 4)
    scr_pool = pool("scr", 2)
    ep_pool = pool("ep", 1)
    recv_pool = pool("recv", 1)
    dram_pool = pool("dram", 1, space="DRAM")
    dse_psum = pool("dsepsum", 1, space="PSUM")
    head_psum = pool("headpsum", 4, space="PSUM")

    # ---- p2p exchange setup: clear sems, pre-generate RDMA descriptors ----
    recv = recv_pool.tile([128, 8 * PK], f32, name="recv", tag="recv")
    pack = recv[:, 0:PK]  # slot 0 doubles as this core's own pack
    if stage == "full":
        # Dummy 4-byte CC AllReduce, triggered first: a collectives-NEFF
        # gets a coordinated multi-core launch (the CC init barrier), which
        # keeps profiled per-core exec spans honest.  Without it the cores
        # launch ms-staggered under NTFF profiling and every span inflates
        # by the stagger (the p2p gate waits for the slowest core).
        cc_in = dram_pool.tile([1, 1], f32, name="cc_in", tag="cc_in")
        cc_out = dram_pool.tile([1, 1], f32, name="cc_out", tag="cc_out",
                                addr_space="Shared")
        zz = ep_pool.tile([1, 1], f32, name="zz", tag="zz")
        nc.vector.memset(zz[:, :], 0.0)
        nc.sync.dma_start(cc_in[:, :], zz[:, :])
        nc.gpsimd.collective_compute(
            "AllReduce",
            ADD,
            replica_groups=[list(range(NCORES))],
            ins=[cc_in[:, :].opt()],
            outs=[cc_out[:, :].opt()],
        )
        rsem = nc.alloc_semaphore("ar_rsem")
        lsem = nc.alloc_semaphore("ar_lsem")
        for k in range(1, 8):
            rdests = [None] * 8
            rdests[k] = (0, k)  # Δrid=0, Δtpb=k: peer tpb = my_tpb ^ k
            nc.gpsimd.remote_dma_broadcast(
                recv[:, k * PK:(k + 1) * PK],
                pack,
                rsem,
                lsem,
                rdests=rdests,
                queue_num=1,
            )

    # ---- constants: one packed DMA ----
    cpk = cpool.tile([128, CW], f32, name="cpk")
    nc.scalar.dma_start(cpk[:, :], x_cpack[:, :])
    ident = cpool.tile([64, 64], f32, name="ident")
    masks.make_identity(nc, ident[:, :])
    zeros = cpool.tile([128, D], f32, name="zeros")
    nc.vector.memset(zeros[:, :], 0.0)


    def leaky(dst, alo_pool=None):
        # in-place leaky relu via mult+max (dst: [p, D] f32 tile ap)
        a = ep_pool.tile([dst.partition_size(), D], f32, name="lk", tag="lk")
        nc.vector.tensor_scalar_mul(a[:, :], dst, SLOPE)
        nc.vector.tensor_max(dst, dst, a[:, :])

    # ---- head precompute (independent of node loop / collective) ----
    # device_feat_embedding.T = leaky(W1 @ norm(dfs).T + b1)  [128, 64]
    dfsT = cpk[0:64, C_DFST:C_DFST + 64]
    mean_f = ep_pool.tile([64, 1], f32, name="mean_f", tag="mean_f")
    nc.vector.tensor_reduce(mean_f[:, :], dfsT, axis=AX, op=ADD)
    nc.vector.tensor_scalar_mul(mean_f[:, :], mean_f[:, :], 1.0 / D)
    sqf = ep_pool.tile([64, D], f32, name="sqf", tag="sqf")
    qf = ep_pool.tile([64, 1], f32, name="qf", tag="qf")
    nc.scalar.activation(sqf[:, :], dfsT, SQUARE)
    nc.vector.tensor_reduce(qf[:, :], sqf[:, :], axis=AX, op=ADD)
    nc.vector.tensor_scalar_mul(qf[:, :], qf[:, :], 1.0 / D)
    varf = ep_pool.tile([64, 1], f32, name="varf", tag="varf")
    nc.vector.tensor_mul(varf[:, :], mean_f[:, :], mean_f[:, :])
    nc.vector.tensor_sub(varf[:, :], qf[:, :], varf[:, :])
    stdf = ep_pool.tile([64, 1], f32, name="stdf", tag="stdf")
    nc.scalar.activation(stdf[:, :], varf[:, :], SQRT)
    nc.vector.tensor_scalar_add(stdf[:, :], stdf[:, :], EPS)
    invf = ep_pool.tile([64, 1], f32, name="invf", tag="invf")
    nc.vector.reciprocal(invf[:, :], stdf[:, :])
    dfsn = ep_pool.tile([64, D], f32, name="dfsn", tag="dfsn")
    nc.vector.tensor_scalar(dfsn[:, :], dfsT, mean_f[:, :], invf[:, :],
                            op0=SUB, op1=MUL)
    psum_dfe = head_psum.tile([H1, D], f32, name="psum_dfe", tag="hp")
    nc.tensor.matmul(psum_dfe[:, :], lhsT=cpk[0:64, C_W1T:C_W1T + H1],
                     rhs=dfsn[:, :], start=True, stop=True)
    dfeT = ep_pool.tile([H1, D], f32, name="dfeT", tag="dfeT")
    nc.scalar.activation(dfeT[:, :], psum_dfe[:, :], IDENT,
                         bias=cpk[:, C_B1:C_B1 + 1])
    leaky(dfeT[:, :])

    # repe = mpnn[pred] broadcast along free axis  [128, 64]
    repe = ep_pool.tile([H1, D], f32, name="repe", tag="repe")
    nc.scalar.activation(repe[:, :], zeros[:, :], IDENT,
                         bias=cpk[:, C_MPRED:C_MPRED + 1])

    # ---- main loop: dse matmuls + state column stats ----
    s_cols = ep_pool.tile([128, NSUP], f32, name="s_cols", tag="s_cols")
    q_cols = ep_pool.tile([128, NSUP], f32, name="q_cols", tag="q_cols")
    psum_dse = dse_psum.tile([D, H1], f32, name="psum_dse", tag="psum_dse")

    # A DMA queue keeps only ~4 packets in flight (~90 GB/s); balance the
    # 14.7MB of loads across the three available queues (SP, ACT, SWDGE):
    #   SP   <- mpnn cols 0:2432        (623KB per super-tile)
    #   ACT  <- mpnn cols 2432:4096 + state cols 0:768    (622KB)
    #   Pool <- mask + state cols 768:2048               (590KB)
    SW = STW // NSUP
    MPC = BPS * H1          # mpnn cols per super-tile = 4096
    MSPL = 17 * 128         # mpnn split point (matmul-block aligned)
    SSPL = 256              # state split point
    for i in range(NSUP):
        mk = mask_pool.tile([128, BPS * D], u8, name="mk", tag="mk")
        nc.gpsimd.dma_start(mk[:, :],
                            x_maskT[:, i * BPS * D:(i + 1) * BPS * D])
        mp = mpnn_pool.tile([128, MPC], bf16, name="mp", tag="mp")
        nc.sync.dma_start(mp[:, 0:MSPL],
                          x_mpnn[:, i * MPC:i * MPC + MSPL])
        nc.scalar.dma_start(mp[:, MSPL:MPC],
                            x_mpnn[:, i * MPC + MSPL:(i + 1) * MPC])
        st = state_pool.tile([128, SW], bf16, name="st", tag="st")
        nc.scalar.dma_start(st[:, 0:SSPL],
                            x_state[:, i * SW:i * SW + SSPL])
        nc.gpsimd.dma_start(st[:, SSPL:SW],
                            x_state[:, i * SW + SSPL:(i + 1) * SW])

        if stage != "dmaonly":
            for j in range(BPS):
                b = i * BPS + j
                nc.tensor.matmul(
                    psum_dse[:, :],
                    lhsT=mk[:, j * D:(j + 1) * D].bitcast(f8e4),
                    rhs=mp[:, j * H1:(j + 1) * H1],
                    start=(b == 0),
                    stop=(b == NBLK - 1),
                )

        nc.vector.tensor_reduce(s_cols[:, i:i + 1], st[:, :], axis=AX, op=ADD)
        scr = scr_pool.tile([128, STW // NSUP], bf16, name="scr", tag="scr")
        nc.scalar.activation(scr[:, :], st[:, :], SQUARE,
                             accum_out=q_cols[:, i:i + 1])

    if stage == "dmaonly":
        o = ep_pool.tile([D, 1], f32, name="odbg", tag="odbg")
        nc.vector.tensor_copy(o[:, :], s_cols[0:D, 0:1])
        nc.sync.dma_start(y_out[:], o[:, 0])
        for p in reversed(ctx_pools):
            p.__exit__(None, None, None)
        return

    # ---- fold per-tile stats, transpose dse, build pack ----
    acc_s = ep_pool.tile([128, 1], f32, name="acc_s", tag="acc_s")
    nc.vector.tensor_reduce(acc_s[:, :], s_cols[:, :], axis=AX, op=ADD)
    acc_q = ep_pool.tile([128, 1], f32, name="acc_q", tag="acc_q")
    nc.vector.tensor_reduce(acc_q[:, :], q_cols[:, :], axis=AX, op=ADD)

    dse_sb = ep_pool.tile([D, H1], f32, name="dse_sb", tag="dse_sb")
    nc.vector.tensor_copy(dse_sb[:, :], psum_dse[:, :])
    psum_T = head_psum.tile([H1, D], f32, name="psum_T", tag="hp")
    nc.tensor.transpose(psum_T[:, :], dse_sb[:, :], ident[:, :])

    pack_copies = [
        nc.vector.tensor_copy(pack[:, 0:64], psum_T[:, :]),
        nc.vector.tensor_copy(pack[:, 64:65], acc_s[:, :]),
        nc.vector.tensor_copy(pack[:, 65:66], acc_q[:, :]),
    ]

    if stage == "loop":
        nc.sync.dma_start(y_out[:], pack[0:D, 0].partition_broadcast(D) if False else pack[0, 0:64])
        for p in reversed(ctx_pools):
            p.__exit__(None, None, None)
        return

    # ---- fire the 7 pre-generated sends; sum the 8 slots on arrival ----
    # signals_writable=[pack] makes the trigger WAW-depend on the pack
    # copies above (the preps were emitted before the pack existed, so the
    # deferred source-read alone does not order the trigger after them).
    if stage == "full":
        trig = nc.gpsimd.trigger_dma(count=None, queue_num=1,
                                     signals_writable=[pack])

    # The gate is pinned between the pack copies and the slot sums with
    # explicit scheduler dependency edges -- it has no data operands, so
    # without them tile is free to float it anywhere in the DVE stream.
    from concourse.tile import add_dep_helper

    if stage == "full":
        gate = nc.vector.wait_ge(rsem, 0)
        for pc in pack_copies:
            add_dep_helper(gate.ins, pc.ins, sync=False,
                           reason="arrival gate after pack")
        hooks["gate"] = gate
        hooks["rsem"] = rsem
        red2 = ep_pool.tile([128, 4 * PK], f32, name="red2", tag="red2")
        add1 = nc.vector.tensor_add(red2[:, :], recv[:, 0:4 * PK],
                                    recv[:, 4 * PK:8 * PK])
        add_dep_helper(add1.ins, gate.ins, sync=False,
                       reason="slot sums after arrival gate")
        hooks["first_read"] = add1
        red1 = ep_pool.tile([128, 2 * PK], f32, name="red1", tag="red1")
        nc.vector.tensor_add(red1[:, :], red2[:, 0:2 * PK],
                             red2[:, 2 * PK:4 * PK])
        red = ep_pool.tile([128, PK], f32, name="red", tag="red")
        nc.vector.tensor_add(red[:, :], red1[:, 0:PK], red1[:, PK:2 * PK])
        # end-of-run reset: sems are zero at NEFF load, never cleared at
        # start (a fast peer's sends could arrive before a slow peer's
        # start-clear and be wiped).  The gate passing implies rsem==14 --
        # every arrival of this run is in -- and the next run's arrivals
        # are >= one loop-time away, so a post-gate clear cannot race.
        cl = nc.gpsimd.sem_clear(rsem)
        add_dep_helper(cl.ins, trig.ins, sync=False, reason="clear after trig")
        add_dep_helper(cl.ins, gate.ins, sync=True, reason="clear after gate")
    else:
        # nocomm debug: use the local pack as "red" (numerically wrong by
        # a factor ~8 on the sums, but exercises everything but the p2p).
        red = ep_pool.tile([128, PK], f32, name="red", tag="red")
        nc.vector.tensor_scalar_mul(red[:, :], recv[:, 0:PK], float(NCORES))

    # ---- replicated MLP head ----
    dseT = red[:, 0:64]          # [128 h1, 64 d] global masked sums

    # state per-feature mean / inv(std+eps): fold halves via PE, then stats
    psum_f = head_psum.tile([64, 2], f32, name="psum_f", tag="hp")
    nc.tensor.matmul(psum_f[:, :], lhsT=cpk[:, C_FOLD:C_FOLD + 64],
                     rhs=red[:, 64:66], start=True, stop=True)
    mean_s = ep_pool.tile([64, 1], f32, name="mean_s", tag="mean_s")
    nc.scalar.activation(mean_s[:, :], psum_f[:, 0:1], IDENT, scale=1.0 / N)
    ex2_s = ep_pool.tile([64, 1], f32, name="ex2_s", tag="ex2_s")
    nc.scalar.activation(ex2_s[:, :], psum_f[:, 1:2], IDENT, scale=1.0 / N)
    var_s = ep_pool.tile([64, 1], f32, name="var_s", tag="var_s")
    nc.vector.tensor_mul(var_s[:, :], mean_s[:, :], mean_s[:, :])
    nc.vector.tensor_sub(var_s[:, :], ex2_s[:, :], var_s[:, :])
    std_s = ep_pool.tile([64, 1], f32, name="std_s", tag="std_s")
    nc.scalar.activation(std_s[:, :], var_s[:, :], SQRT)
    nc.vector.tensor_scalar_add(std_s[:, :], std_s[:, :], EPS)
    inv_s = ep_pool.tile([64, 1], f32, name="inv_s", tag="inv_s")
    nc.vector.reciprocal(inv_s[:, :], std_s[:, :])

    # rep_latent.T = leaky(W2 @ norm(state[pred]) + b2) for all D columns
    xn = ep_pool.tile([F, 1], f32, name="xn", tag="xn")
    nc.vector.tensor_scalar(xn[:, :], cpk[0:64, C_SPRED:C_SPRED + 1],
                            mean_s[:, :], inv_s[:, :], op0=SUB, op1=MUL)
    xn_b = ep_pool.tile([F, D], f32, name="xn_b", tag="xn_b")
    nc.scalar.activation(xn_b[:, :], zeros[0:F, :], IDENT, bias=xn[:, :])
    psum_repl = head_psum.tile([H1, D], f32, name="psum_repl", tag="hp")
    nc.tensor.matmul(psum_repl[:, :], lhsT=cpk[0:64, C_W2T:C_W2T + H1],
                     rhs=xn_b[:, :], start=True, stop=True)
    repl = ep_pool.tile([H1, D], f32, name="repl", tag="repl")
    nc.scalar.activation(repl[:, :], psum_repl[:, :], IDENT,
                         bias=cpk[:, C_B2:C_B2 + 1])
    leaky(repl[:, :])

    # dse normalization over the device (free) axis
    mean_d = ep_pool.tile([H1, 1], f32, name="mean_d", tag="mean_d")
    nc.vector.tensor_reduce(mean_d[:, :], dseT, axis=AX, op=ADD)
    nc.vector.tensor_scalar_mul(mean_d[:, :], mean_d[:, :], 1.0 / D)
    sqd = ep_pool.tile([H1, D], f32, name="sqd", tag="sqd")
    qd = ep_pool.tile([H1, 1], f32, name="qd", tag="qd")
    nc.scalar.activation(sqd[:, :], dseT, SQUARE)
    nc.vector.tensor_reduce(qd[:, :], sqd[:, :], axis=AX, op=ADD)
    nc.vector.tensor_scalar_mul(qd[:, :], qd[:, :], 1.0 / D)
    vard = ep_pool.tile([H1, 1], f32, name="vard", tag="vard")
    nc.vector.tensor_mul(vard[:, :], mean_d[:, :], mean_d[:, :])
    nc.vector.tensor_sub(vard[:, :], qd[:, :], vard[:, :])
    stdd = ep_pool.tile([H1, 1], f32, name="stdd", tag="stdd")
    nc.scalar.activation(stdd[:, :], vard[:, :], SQRT)
    nc.vector.tensor_scalar_add(stdd[:, :], stdd[:, :], EPS)
    invd = ep_pool.tile([H1, 1], f32, name="invd", tag="invd")
    nc.vector.reciprocal(invd[:, :], stdd[:, :])
    dsen = ep_pool.tile([H1, D], f32, name="dsen", tag="dsen")
    nc.vector.tensor_scalar(dsen[:, :], dseT, mean_d[:, :], invd[:, :],
                            op0=SUB, op1=MUL)

    # h.T = leaky(W3 @ concat.T + b3): 4 accumulated chunks
    psum_h = head_psum.tile([H2, D], f32, name="psum_h", tag="hp")
    chunks = [dfeT[:, :], repl[:, :], repe[:, :], dsen[:, :]]
    for k in range(4):
        nc.tensor.matmul(psum_h[:, :],
                         lhsT=cpk[:, C_W3T + k * H2:C_W3T + (k + 1) * H2],
                         rhs=chunks[k], start=(k == 0), stop=(k == 3))
    hT = ep_pool.tile([H2, D], f32, name="hT", tag="hT")
    nc.scalar.activation(hT[:, :], psum_h[:, :], IDENT,
                         bias=cpk[0:64, C_B3:C_B3 + 1])
    leaky(hT[:, :])

    # output[d] = sum_j hT[j, d] * W4[0, j] + b4
    psum_o = head_psum.tile([D, 1], f32, name="psum_o", tag="hp")
    nc.tensor.matmul(psum_o[:, :], lhsT=hT[:, :],
                     rhs=cpk[0:64, C_W4T:C_W4T + 1], start=True, stop=True)
    out_sb = ep_pool.tile([D, 1], f32, name="out_sb", tag="out_sb")
    nc.scalar.activation(out_sb[:, :], psum_o[:, :], IDENT,
                         bias=cpk[0:64, C_B4:C_B4 + 1])
    nc.sync.dma_start(y_out[:], out_sb[:, 0])

    for p in reversed(ctx_pools):
        p.__exit__(None, None, None)


_compiled = None


def _get_compiled():
    global _compiled
    if _compiled is None:
        _compiled = build_program()
    return _compiled


def make_in_maps(inputs):
    state = np.ascontiguousarray(np.asarray(inputs["state"], dtype=np.float32))
    dfs = np.asarray(inputs["device_feat_state"], dtype=np.float32)
    mpnn = np.ascontiguousarray(
        np.asarray(inputs["mpnn_forward"], dtype=np.float32))
    W1 = np.asarray(inputs["W1"], dtype=np.float32)
    b1 = np.asarray(inputs["b1"], dtype=np.float32)
    W2 = np.asarray(inputs["W2"], dtype=np.float32)
    b2 = np.asarray(inputs["b2"], dtype=np.float32)
    W3 = np.asarray(inputs["W3"], dtype=np.float32)
    b3 = np.asarray(inputs["b3"], dtype=np.float32)
    W4 = np.asarray(inputs["W4"], dtype=np.float32)
    b4 = np.asarray(inputs["b4"], dtype=np.float32)
    mask = np.asarray(inputs["device_assign_state"])
    assert mask.dtype == np.int32
    pred = int(np.asarray(inputs["pred_node"]))

    cpack = np.zeros((128, CW), np.float32)
    cpack[0:64, C_DFST:C_DFST + 64] = np.pad(dfs.T, ((0, 64 - DF), (0, 0)))
    cpack[0:64, C_W1T:C_W1T + H1] = np.pad(W1.T, ((0, 64 - DF), (0, 0)))
    cpack[0:64, C_W2T:C_W2T + H1] = W2.T
    cpack[:, C_W3T:C_W3T + 256] = (
        W3.T.reshape(4, H1, H2).transpose(1, 0, 2).reshape(H1, 4 * H2))
    cpack[:, C_B1] = b1
    cpack[:, C_B2] = b2
    cpack[0:64, C_B3] = b3
    cpack[0:64, C_W4T] = W4[0]
    cpack[0:64, C_B4] = b4[0]
    cpack[0:64, C_SPRED] = state[pred]
    cpack[:, C_MPRED] = mpnn[pred]
    cpack[:, C_FOLD:C_FOLD + 64] = np.tile(np.eye(64, dtype=np.float32),
                                           (2, 1))

    in_maps = []
    for c in range(NCORES):
        sl = slice(c * NSH, (c + 1) * NSH)
        mp = mpnn[sl]  # [NSH, H1]
        mp_p = np.ascontiguousarray(
            mp.reshape(NBLK, 128, H1).transpose(1, 0, 2).reshape(128, MPW)
        ).astype(ml_dtypes.bfloat16)
        st = state[sl]  # [NSH, F]
        st_p = np.ascontiguousarray(
            np.concatenate([st[:NSH // 2].T, st[NSH // 2:].T], axis=0)
        ).astype(ml_dtypes.bfloat16)
        mk = mask[:, sl]  # [D, NSH] int 0/1
        mk_p = (
            mk.T.reshape(NBLK, 128, D).transpose(1, 0, 2).reshape(128, MKW)
            .astype(np.uint8) * np.uint8(0x38)  # fp8 e4m3 1.0
        )
        in_maps.append({
            "x_mpnn": mp_p,
            "x_state": st_p,
            "x_maskT": np.ascontiguousarray(mk_p),
            "x_cpack": cpack,
        })
    return in_maps


def kernel(**inputs) -> np.ndarray:
    nc = _get_compiled()
    in_maps = make_in_maps(inputs)
    res = run_bass_kernel_spmd(nc, in_maps, core_ids=list(range(NCORES)))
    return np.asarray(res.results[0]["y_out"], dtype=np.float32)


# revision 25
# speedup vs baseline: 1.0498x; 1.0498x over previous
"""Trainium2 Bass kernel for nn_Device_Policy (segment_reduce).

Strategy: shard the node axis N across 8 NeuronCores.  Host-side the
inputs are repacked into DMA/PE-friendly layouts and compressed:

  - mpnn_forward -> bf16, packed [128, 32768] so partition p holds node
    b*128+p of the shard for block b (free offset b*128+h).
  - state -> bf16, packed [128, 16384]: features on partitions, the two
    node-halves of the shard stacked (partition f = feat f of nodes
    0..16383, partition 64+f = feat f of nodes 16384..32767).  Column
    sums / sums-of-squares then reduce along the free axis.
  - mask -> fp8 e4m3 bytes (0x00/0x38), pre-transposed and packed
    [128, 16384] with partition p = node b*128+p, free b*64+d.

Per core the masked segment sum dse[d, h1] accumulates in one PSUM bank
over 256 matmuls (lhsT = fp8 mask block [128n, 64d] stationary, rhs =
bf16 mpnn block [128n, 128h] moving).  The [128, 66] pack (dse.T +
state sums) is exchanged via raw peer-to-peer SWDGE remote DMA (XOR
slotting: core c sends its pack to peer c^k's slot k, k=1..7) instead
of the CC-library AllReduce (which costs ~35us for 34KB); each core
sums the 8 slots and runs the small replicated MLP head.
"""

import sys

if "/opt/trn_rl_repo" not in sys.path:
    sys.path.insert(0, "/opt/trn_rl_repo")

import numpy as np
import ml_dtypes

import concourse.bacc as bacc
import concourse.bass as bass
import concourse.mybir as mybir
import concourse.tile as tile
from concourse import masks
from concourse.bass_utils import run_bass_kernel_spmd

NCORES = 8
N = 262144
F = 64
D = 64
DF = 32
H1 = 128
H2 = 64
NSH = N // NCORES          # nodes per core = 32768
NBLK = NSH // 128          # 128-node matmul blocks per core = 256
NSUP = 8                   # DMA super-tiles
BPS = NBLK // NSUP         # blocks per super-tile = 32
MPW = NSH * H1 // 128      # mpnn pack free width = 32768
STW = NSH * F // 128       # state pack free width = 16384
MKW = NSH * D // 128       # mask pack free width = 16384
EPS = 1e-6
SLOPE = 0.1
PK = 66                    # pack columns: 64 dse.T + ssum + ssq

f32 = mybir.dt.float32
bf16 = mybir.dt.bfloat16
f8e4 = mybir.dt.float8e4
u8 = mybir.dt.uint8
ADD = mybir.AluOpType.add
MUL = mybir.AluOpType.mult
SUB = mybir.AluOpType.subtract
AX = mybir.AxisListType.X
IDENT = mybir.ActivationFunctionType.Identity
SQUARE = mybir.ActivationFunctionType.Square
SQRT = mybir.ActivationFunctionType.Sqrt

# cpack free-column layout (f32 columns in one packed const tensor)
C_DFST = 0      # [64, 64]   device_feat_state.T (padded 32->64 rows)
C_W1T = 64      # [64, 128]  W1.T padded
C_W2T = 192     # [64, 128]  W2.T
C_W3T = 320     # [128, 256] W3.T, 4 chunks of [128, 64]
C_B1 = 576      # [128, 1]
C_B2 = 577      # [128, 1]
C_B3 = 578      # [64, 1]
C_W4T = 579     # [64, 1]
C_B4 = 580      # [64, 1]
C_SPRED = 581   # [64, 1]
C_MPRED = 582   # [128, 1]
C_FOLD = 583    # [128, 64]  vertical-stacked I64: folds p and p+64
CW = 648


def build_program(stage="full"):
    nc = bacc.Bacc(
        "TRN2",
        target_bir_lowering=False,
        debug=False,
        enable_asserts=False,
        num_devices=NCORES,
        num_swdge_queues=4,
    )

    x_mpnn = nc.dram_tensor("x_mpnn", [128, MPW], bf16, kind="ExternalInput")
    x_state = nc.dram_tensor("x_state", [128, STW], bf16, kind="ExternalInput")
    x_maskT = nc.dram_tensor("x_maskT", [128, MKW], u8, kind="ExternalInput")
    x_cpack = nc.dram_tensor("x_cpack", [128, CW], f32, kind="ExternalInput")
    y_out = nc.dram_tensor("y_out", [D], f32, kind="ExternalOutput")

    hooks = {}
    with tile.TileContext(nc) as tc:
        emit(nc, tc, x_mpnn, x_state, x_maskT, x_cpack, y_out, hooks,
             stage=stage)

    # The arrival gate (rsem >= 14, incremented only by the 7 peers' remote
    # DMAs) is attached AFTER tile scheduling: the scheduling-pass CoreSim
    # has no peers, so an emit-time wait would deadlock it.  The gate is a
    # standalone DVE EventSemaphore (2 wait slots) emitted with a trivially
    # true rsem>=0 wait to pin its stream position; the real >=14 wait is
    # added here, before codegen.
    if stage == "full":
        import bass_rust
        gate_ins = hooks["gate"].ins
        bass_rust.wait_op(gate_ins, hooks["rsem"], 14, "sem-ge", True)

    nc.compile()

    if stage == "full":
        # Safety: the gate must precede the first remote-slot read (the
        # slot-sum add) in the final DVE stream, else the exchange races.
        seen_gate = False
        for f in nc.m.functions:
            for bb in f.blocks:
                for i in bb.instructions:
                    if i.name == gate_ins.name:
                        seen_gate = True
                    if i.name == hooks["first_read"].ins.name:
                        assert seen_gate, "gate scheduled after slot sums"
        assert seen_gate, "arrival gate missing from compiled program"
    return nc


def emit(nc, tc, x_mpnn, x_state, x_maskT, x_cpack, y_out, hooks,
         stage="full"):
    ctx_pools = []

    def pool(name, bufs, space="SBUF"):
        p = tc.tile_pool(name=name, bufs=bufs, space=space)
        ctx_pools.append(p)
        return p.__enter__()

    cpool = pool("const", 1)
    mpnn_pool = pool("mpnn", 5)
    mask_pool = pool("maskp", 4)
    state_pool = pool("state", 4)
    scr_pool = pool("scr", 2)
    ep_pool = pool("ep", 1)
    recv_pool = pool("recv", 1)
    dram_pool = pool("dram", 1, space="DRAM")
    dse_psum = pool("dsepsum", 1, space="PSUM")
    head_psum = pool("headpsum", 4, space="PSUM")

    # ---- p2p exchange setup: clear sems, pre-generate RDMA descriptors ----
    recv = recv_pool.tile([128, 8 * PK], f32, name="recv", tag="recv")
    pack = recv[:, 0:PK]  # slot 0 doubles as this core's own pack
    if stage == "full":
        # Dummy 4-byte CC AllReduce, triggered first: a collectives-NEFF
        # gets a coordinated multi-core launch (the CC init barrier), which
        # keeps profiled per-core exec spans honest.  Without it the cores
        # launch ms-staggered under NTFF profiling and every span inflates
        # by the stagger (the p2p gate waits for the slowest core).
        cc_in = dram_pool.tile([1, 1], f32, name="cc_in", tag="cc_in")
        cc_out = dram_pool.tile([1, 1], f32, name="cc_out", tag="cc_out",
                                addr_space="Shared")
        zz = ep_pool.tile([1, 1], f32, name="zz", tag="zz")
        nc.vector.memset(zz[:, :], 0.0)
        nc.sync.dma_start(cc_in[:, :], zz[:, :])
        nc.gpsimd.collective_compute(
            "AllReduce",
            ADD,
            replica_groups=[list(range(NCORES))],
            ins=[cc_in[:, :].opt()],
            outs=[cc_out[:, :].opt()],
        )
        rsem = nc.alloc_semaphore("ar_rsem")
        lsem = nc.alloc_semaphore("ar_lsem")
        for k in range(1, 8):
            rdests = [None] * 8
            rdests[k] = (0, k)  # Δrid=0, Δtpb=k: peer tpb = my_tpb ^ k
            nc.gpsimd.remote_dma_broadcast(
                recv[:, k * PK:(k + 1) * PK],
                pack,
                rsem,
                lsem,
                rdests=rdests,
                queue_num=1,
            )

    # ---- constants: one packed DMA ----
    cpk = cpool.tile([128, CW], f32, name="cpk")
    nc.scalar.dma_start(cpk[:, :], x_cpack[:, :])
    ident = cpool.tile([64, 64], f32, name="ident")
    masks.make_identity(nc, ident[:, :])
    zeros = cpool.tile([128, D], f32, name="zeros")
    nc.vector.memset(zeros[:, :], 0.0)


    def leaky(dst, alo_pool=None):
        # in-place leaky relu via mult+max (dst: [p, D] f32 tile ap)
        a = ep_pool.tile([dst.partition_size(), D], f32, name="lk", tag="lk")
        nc.vector.tensor_scalar_mul(a[:, :], dst, SLOPE)
        nc.vector.tensor_max(dst, dst, a[:, :])

    # ---- head precompute (independent of node loop / collective) ----
    # device_feat_embedding.T = leaky(W1 @ norm(dfs).T + b1)  [128, 64]
    dfsT = cpk[0:64, C_DFST:C_DFST + 64]
    mean_f = ep_pool.tile([64, 1], f32, name="mean_f", tag="mean_f")
    nc.vector.tensor_reduce(mean_f[:, :], dfsT, axis=AX, op=ADD)
    nc.vector.tensor_scalar_mul(mean_f[:, :], mean_f[:, :], 1.0 / D)
    sqf = ep_pool.tile([64, D], f32, name="sqf", tag="sqf")
    qf = ep_pool.tile([64, 1], f32, name="qf", tag="qf")
    nc.scalar.activation(sqf[:, :], dfsT, SQUARE)
    nc.vector.tensor_reduce(qf[:, :], sqf[:, :], axis=AX, op=ADD)
    nc.vector.tensor_scalar_mul(qf[:, :], qf[:, :], 1.0 / D)
    varf = ep_pool.tile([64, 1], f32, name="varf", tag="varf")
    nc.vector.tensor_mul(varf[:, :], mean_f[:, :], mean_f[:, :])
    nc.vector.tensor_sub(varf[:, :], qf[:, :], varf[:, :])
    stdf = ep_pool.tile([64, 1], f32, name="stdf", tag="stdf")
    nc.scalar.activation(stdf[:, :], varf[:, :], SQRT)
    nc.vector.tensor_scalar_add(stdf[:, :], stdf[:, :], EPS)
    invf = ep_pool.tile([64, 1], f32, name="invf", tag="invf")
    nc.vector.reciprocal(invf[:, :], stdf[:, :])
    dfsn = ep_pool.tile([64, D], f32, name="dfsn", tag="dfsn")
    nc.vector.tensor_scalar(dfsn[:, :], dfsT, mean_f[:, :], invf[:, :],
                            op0=SUB, op1=MUL)
    psum_dfe = head_psum.tile([H1, D], f32, name="psum_dfe", tag="hp")
    nc.tensor.matmul(psum_dfe[:, :], lhsT=cpk[0:64, C_W1T:C_W1T + H1],
                     rhs=dfsn[:, :], start=True, stop=True)
    dfeT = ep_pool.tile([H1, D], f32, name="dfeT", tag="dfeT")
    nc.scalar.activation(dfeT[:, :], psum_dfe[:, :], IDENT,
                         bias=cpk[:, C_B1:C_B1 + 1])
    leaky(dfeT[:, :])

    # repe = mpnn[pred] broadcast along free axis  [128, 64]
    repe = ep_pool.tile([H1, D], f32, name="repe", tag="repe")
    nc.scalar.activation(repe[:, :], zeros[:, :], IDENT,
                         bias=cpk[:, C_MPRED:C_MPRED + 1])

    # ---- main loop: dse matmuls + state column stats ----
    s_cols = ep_pool.tile([128, NSUP], f32, name="s_cols", tag="s_cols")
    q_cols = ep_pool.tile([128, NSUP], f32, name="q_cols", tag="q_cols")
    psum_dse = dse_psum.tile([D, H1], f32, name="psum_dse", tag="psum_dse")

    # Per-queue sustained rates measured: SP ~38 B/ns, ACT ~80, SWDGE-q0
    # ~133.  Split each super-tile's 1835KB proportionally:
    #   SP   <- mpnn cols 0:1024            (262KB)
    #   ACT  <- mpnn cols 1024:3328         (590KB)
    #   Pool <- mask + mpnn 3328:4096 + state (977KB)
    SW = STW // NSUP
    MPC = BPS * H1          # mpnn cols per super-tile = 4096
    MA = 8 * 128
    MB = 26 * 128
    for i in range(NSUP):
        mk = mask_pool.tile([128, BPS * D], u8, name="mk", tag="mk")
        nc.gpsimd.dma_start(mk[:, :],
                            x_maskT[:, i * BPS * D:(i + 1) * BPS * D])
        mp = mpnn_pool.tile([128, MPC], bf16, name="mp", tag="mp")
        nc.sync.dma_start(mp[:, 0:MA],
                          x_mpnn[:, i * MPC:i * MPC + MA])
        nc.scalar.dma_start(mp[:, MA:MB],
                            x_mpnn[:, i * MPC + MA:i * MPC + MB])
        nc.gpsimd.dma_start(mp[:, MB:MPC],
                            x_mpnn[:, i * MPC + MB:(i + 1) * MPC])
        st = state_pool.tile([128, SW], bf16, name="st", tag="st")
        nc.gpsimd.dma_start(st[:, :],
                            x_state[:, i * SW:(i + 1) * SW])

        if stage != "dmaonly":
            for j in range(BPS):
                b = i * BPS + j
                nc.tensor.matmul(
                    psum_dse[:, :],
                    lhsT=mk[:, j * D:(j + 1) * D].bitcast(f8e4),
                    rhs=mp[:, j * H1:(j + 1) * H1],
                    start=(b == 0),
                    stop=(b == NBLK - 1),
                )

        nc.vector.tensor_reduce(s_cols[:, i:i + 1], st[:, :], axis=AX, op=ADD)
        scr = scr_pool.tile([128, STW // NSUP], bf16, name="scr", tag="scr")
        nc.scalar.activation(scr[:, :], st[:, :], SQUARE,
                             accum_out=q_cols[:, i:i + 1])

    if stage == "dmaonly":
        o = ep_pool.tile([D, 1], f32, name="odbg", tag="odbg")
        nc.vector.tensor_copy(o[:, :], s_cols[0:D, 0:1])
        nc.sync.dma_start(y_out[:], o[:, 0])
        for p in reversed(ctx_pools):
            p.__exit__(None, None, None)
        return

    # ---- fold per-tile stats, transpose dse, build pack ----
    acc_s = ep_pool.tile([128, 1], f32, name="acc_s", tag="acc_s")
    nc.vector.tensor_reduce(acc_s[:, :], s_cols[:, :], axis=AX, op=ADD)
    acc_q = ep_pool.tile([128, 1], f32, name="acc_q", tag="acc_q")
    nc.vector.tensor_reduce(acc_q[:, :], q_cols[:, :], axis=AX, op=ADD)

    dse_sb = ep_pool.tile([D, H1], f32, name="dse_sb", tag="dse_sb")
    nc.vector.tensor_copy(dse_sb[:, :], psum_dse[:, :])
    psum_T = head_psum.tile([H1, D], f32, name="psum_T", tag="hp")
    nc.tensor.transpose(psum_T[:, :], dse_sb[:, :], ident[:, :])

    pack_copies = [
        nc.vector.tensor_copy(pack[:, 0:64], psum_T[:, :]),
        nc.vector.tensor_copy(pack[:, 64:65], acc_s[:, :]),
        nc.vector.tensor_copy(pack[:, 65:66], acc_q[:, :]),
    ]

    if stage == "loop":
        nc.sync.dma_start(y_out[:], pack[0:D, 0].partition_broadcast(D) if False else pack[0, 0:64])
        for p in reversed(ctx_pools):
            p.__exit__(None, None, None)
        return

    # ---- fire the 7 pre-generated sends; sum the 8 slots on arrival ----
    # signals_writable=[pack] makes the trigger WAW-depend on the pack
    # copies above (the preps were emitted before the pack existed, so the
    # deferred source-read alone does not order the trigger after them).
    if stage == "full":
        trig = nc.gpsimd.trigger_dma(count=None, queue_num=1,
                                     signals_writable=[pack])

    # The gate is pinned between the pack copies and the slot sums with
    # explicit scheduler dependency edges -- it has no data operands, so
    # without them tile is free to float it anywhere in the DVE stream.
    from concourse.tile import add_dep_helper

    if stage == "full":
        gate = nc.vector.wait_ge(rsem, 0)
        for pc in pack_copies:
            add_dep_helper(gate.ins, pc.ins, sync=False,
                           reason="arrival gate after pack")
        hooks["gate"] = gate
        hooks["rsem"] = rsem
        red2 = ep_pool.tile([128, 4 * PK], f32, name="red2", tag="red2")
        add1 = nc.vector.tensor_add(red2[:, :], recv[:, 0:4 * PK],
                                    recv[:, 4 * PK:8 * PK])
        add_dep_helper(add1.ins, gate.ins, sync=False,
                       reason="slot sums after arrival gate")
        hooks["first_read"] = add1
        red1 = ep_pool.tile([128, 2 * PK], f32, name="red1", tag="red1")
        nc.vector.tensor_add(red1[:, :], red2[:, 0:2 * PK],
                             red2[:, 2 * PK:4 * PK])
        red = ep_pool.tile([128, PK], f32, name="red", tag="red")
        nc.vector.tensor_add(red[:, :], red1[:, 0:PK], red1[:, PK:2 * PK])
        # end-of-run reset: sems are zero at NEFF load, never cleared at
        # start (a fast peer's sends could arrive before a slow peer's
        # start-clear and be wiped).  The gate passing implies rsem==14 --
        # every arrival of this run is in -- and the next run's arrivals
        # are >= one loop-time away, so a post-gate clear cannot race.
        cl = nc.gpsimd.sem_clear(rsem)
        add_dep_helper(cl.ins, trig.ins, sync=False, reason="clear after trig")
        add_dep_helper(cl.ins, gate.ins, sync=True, reason="clear after gate")
    else:
        # nocomm debug: use the local pack as "red" (numerically wrong by
        # a factor ~8 on the sums, but exercises everything but the p2p).
        red = ep_pool.tile([128, PK], f32, name="red", tag="red")
        nc.vector.tensor_scalar_mul(red[:, :], recv[:, 0:PK], float(NCORES))

    # ---- replicated MLP head ----
    dseT = red[:, 0:64]          # [128 h1, 64 d] global masked sums

    # state per-feature mean / inv(std+eps): fold halves via PE, then stats
    psum_f = head_psum.tile([64, 2], f32, name="psum_f", tag="hp")
    nc.tensor.matmul(psum_f[:, :], lhsT=cpk[:, C_FOLD:C_FOLD + 64],
                     rhs=red[:, 64:66], start=True, stop=True)
    mean_s = ep_pool.tile([64, 1], f32, name="mean_s", tag="mean_s")
    nc.scalar.activation(mean_s[:, :], psum_f[:, 0:1], IDENT, scale=1.0 / N)
    ex2_s = ep_pool.tile([64, 1], f32, name="ex2_s", tag="ex2_s")
    nc.scalar.activation(ex2_s[:, :], psum_f[:, 1:2], IDENT, scale=1.0 / N)
    var_s = ep_pool.tile([64, 1], f32, name="var_s", tag="var_s")
    nc.vector.tensor_mul(var_s[:, :], mean_s[:, :], mean_s[:, :])
    nc.vector.tensor_sub(var_s[:, :], ex2_s[:, :], var_s[:, :])
    std_s = ep_pool.tile([64, 1], f32, name="std_s", tag="std_s")
    nc.scalar.activation(std_s[:, :], var_s[:, :], SQRT)
    nc.vector.tensor_scalar_add(std_s[:, :], std_s[:, :], EPS)
    inv_s = ep_pool.tile([64, 1], f32, name="inv_s", tag="inv_s")
    nc.vector.reciprocal(inv_s[:, :], std_s[:, :])

    # rep_latent.T = leaky(W2 @ norm(state[pred]) + b2) for all D columns
    xn = ep_pool.tile([F, 1], f32, name="xn", tag="xn")
    nc.vector.tensor_scalar(xn[:, :], cpk[0:64, C_SPRED:C_SPRED + 1],
                            mean_s[:, :], inv_s[:, :], op0=SUB, op1=MUL)
    xn_b = ep_pool.tile([F, D], f32, name="xn_b", tag="xn_b")
    nc.scalar.activation(xn_b[:, :], zeros[0:F, :], IDENT, bias=xn[:, :])
    psum_repl = head_psum.tile([H1, D], f32, name="psum_repl", tag="hp")
    nc.tensor.matmul(psum_repl[:, :], lhsT=cpk[0:64, C_W2T:C_W2T + H1],
                     rhs=xn_b[:, :], start=True, stop=True)
    repl = ep_pool.tile([H1, D], f32, name="repl", tag="repl")
    nc.scalar.activation(repl[:, :], psum_repl[:, :], IDENT,
                         bias=cpk[:, C_B2:C_B2 + 1])
    leaky(repl[:, :])

    # dse normalization over the device (free) axis
    mean_d = ep_pool.tile([H1, 1], f32, name="mean_d", tag="mean_d")
    nc.vector.tensor_reduce(mean_d[:, :], dseT, axis=AX, op=ADD)
    nc.vector.tensor_scalar_mul(mean_d[:, :], mean_d[:, :], 1.0 / D)
    sqd = ep_pool.tile([H1, D], f32, name="sqd", tag="sqd")
    qd = ep_pool.tile([H1, 1], f32, name="qd", tag="qd")
    nc.scalar.activation(sqd[:, :], dseT, SQUARE)
    nc.vector.tensor_reduce(qd[:, :], sqd[:, :], axis=AX, op=ADD)
    nc.vector.tensor_scalar_mul(qd[:, :], qd[:, :], 1.0 / D)
    vard = ep_pool.tile([H1, 1], f32, name="vard", tag="vard")
    nc.vector.tensor_mul(vard[:, :], mean_d[:, :], mean_d[:, :])
    nc.vector.tensor_sub(vard[:, :], qd[:, :], vard[:, :])
    stdd = ep_pool.tile([H1, 1], f32, name="stdd", tag="stdd")
    nc.scalar.activation(stdd[:, :], vard[:, :], SQRT)
    nc.vector.tensor_scalar_add(stdd[:, :], stdd[:, :], EPS)
    invd = ep_pool.tile([H1, 1], f32, name="invd", tag="invd")
    nc.vector.reciprocal(invd[:, :], stdd[:, :])
    dsen = ep_pool.tile([H1, D], f32, name="dsen", tag="dsen")
    nc.vector.tensor_scalar(dsen[:, :], dseT, mean_d[:, :], invd[:, :],
                            op0=SUB, op1=MUL)

    # h.T = leaky(W3 @ concat.T + b3): 4 accumulated chunks
    psum_h = head_psum.tile([H2, D], f32, name="psum_h", tag="hp")
    chunks = [dfeT[:, :], repl[:, :], repe[:, :], dsen[:, :]]
    for k in range(4):
        nc.tensor.matmul(psum_h[:, :],
                         lhsT=cpk[:, C_W3T + k * H2:C_W3T + (k + 1) * H2],
                         rhs=chunks[k], start=(k == 0), stop=(k == 3))
    hT = ep_pool.tile([H2, D], f32, name="hT", tag="hT")
    nc.scalar.activation(hT[:, :], psum_h[:, :], IDENT,
                         bias=cpk[0:64, C_B3:C_B3 + 1])
    leaky(hT[:, :])

    # output[d] = sum_j hT[j, d] * W4[0, j] + b4
    psum_o = head_psum.tile([D, 1], f32, name="psum_o", tag="hp")
    nc.tensor.matmul(psum_o[:, :], lhsT=hT[:, :],
                     rhs=cpk[0:64, C_W4T:C_W4T + 1], start=True, stop=True)
    out_sb = ep_pool.tile([D, 1], f32, name="out_sb", tag="out_sb")
    nc.scalar.activation(out_sb[:, :], psum_o[:, :], IDENT,
                         bias=cpk[0:64, C_B4:C_B4 + 1])
    nc.sync.dma_start(y_out[:], out_sb[:, 0])

    for p in reversed(ctx_pools):
        p.__exit__(None, None, None)


_compiled = None


def _get_compiled():
    global _compiled
    if _compiled is None:
        _compiled = build_program()
    return _compiled


def make_in_maps(inputs):
    state = np.ascontiguousarray(np.asarray(inputs["state"], dtype=np.float32))
    dfs = np.asarray(inputs["device_feat_state"], dtype=np.float32)
    mpnn = np.ascontiguousarray(
        np.asarray(inputs["mpnn_forward"], dtype=np.float32))
    W1 = np.asarray(inputs["W1"], dtype=np.float32)
    b1 = np.asarray(inputs["b1"], dtype=np.float32)
    W2 = np.asarray(inputs["W2"], dtype=np.float32)
    b2 = np.asarray(inputs["b2"], dtype=np.float32)
    W3 = np.asarray(inputs["W3"], dtype=np.float32)
    b3 = np.asarray(inputs["b3"], dtype=np.float32)
    W4 = np.asarray(inputs["W4"], dtype=np.float32)
    b4 = np.asarray(inputs["b4"], dtype=np.float32)
    mask = np.asarray(inputs["device_assign_state"])
    assert mask.dtype == np.int32
    pred = int(np.asarray(inputs["pred_node"]))

    cpack = np.zeros((128, CW), np.float32)
    cpack[0:64, C_DFST:C_DFST + 64] = np.pad(dfs.T, ((0, 64 - DF), (0, 0)))
    cpack[0:64, C_W1T:C_W1T + H1] = np.pad(W1.T, ((0, 64 - DF), (0, 0)))
    cpack[0:64, C_W2T:C_W2T + H1] = W2.T
    cpack[:, C_W3T:C_W3T + 256] = (
        W3.T.reshape(4, H1, H2).transpose(1, 0, 2).reshape(H1, 4 * H2))
    cpack[:, C_B1] = b1
    cpack[:, C_B2] = b2
    cpack[0:64, C_B3] = b3
    cpack[0:64, C_W4T] = W4[0]
    cpack[0:64, C_B4] = b4[0]
    cpack[0:64, C_SPRED] = state[pred]
    cpack[:, C_MPRED] = mpnn[pred]
    cpack[:, C_FOLD:C_FOLD + 64] = np.tile(np.eye(64, dtype=np.float32),
                                           (2, 1))

    in_maps = []
    for c in range(NCORES):
        sl = slice(c * NSH, (c + 1) * NSH)
        mp = mpnn[sl]  # [NSH, H1]
        mp_p = np.ascontiguousarray(
            mp.reshape(NBLK, 128, H1).transpose(1, 0, 2).reshape(128, MPW)
        ).astype(ml_dtypes.bfloat16)
        st = state[sl]  # [NSH, F]
        st_p = np.ascontiguousarray(
            np.concatenate([st[:NSH // 2].T, st[NSH // 2:].T], axis=0)
        ).astype(ml_dtypes.bfloat16)
        mk = mask[:, sl]  # [D, NSH] int 0/1
        mk_p = (
            mk.T.reshape(NBLK, 128, D).transpose(1, 0, 2).reshape(128, MKW)
            .astype(np.uint8) * np.uint8(0x38)  # fp8 e4m3 1.0
        )
        in_maps.append({
            "x_mpnn": mp_p,
            "x_state": st_p,
            "x_maskT": np.ascontiguousarray(mk_p),
            "x_cpack": cpack,
        })
    return in_maps


def kernel(**inputs) -> np.ndarray:
    nc = _get_compiled()
    in_maps = make_in_maps(inputs)
    res = run_bass_kernel_spmd(nc, in_maps, core_ids=list(range(NCORES)))
    return np.asarray(res.results[0]["y_out"], dtype=np.float32)


# revision 26
# speedup vs baseline: 1.0600x; 1.0097x over previous
"""Trainium2 Bass kernel for nn_Device_Policy (segment_reduce).

Strategy: shard the node axis N across 8 NeuronCores.  Host-side the
inputs are repacked into DMA/PE-friendly layouts and compressed:

  - mpnn_forward -> bf16, packed [128, 32768] so partition p holds node
    b*128+p of the shard for block b (free offset b*128+h).
  - state -> bf16, packed [128, 16384]: features on partitions, the two
    node-halves of the shard stacked (partition f = feat f of nodes
    0..16383, partition 64+f = feat f of nodes 16384..32767).  Column
    sums / sums-of-squares then reduce along the free axis.
  - mask -> fp8 e4m3 bytes (0x00/0x38), pre-transposed and packed
    [128, 16384] with partition p = node b*128+p, free b*64+d.

Per core the masked segment sum dse[d, h1] accumulates in one PSUM bank
over 256 matmuls (lhsT = fp8 mask block [128n, 64d] stationary, rhs =
bf16 mpnn block [128n, 128h] moving).  The [128, 66] pack (dse.T +
state sums) is exchanged via raw peer-to-peer SWDGE remote DMA (XOR
slotting: core c sends its pack to peer c^k's slot k, k=1..7) instead
of the CC-library AllReduce (which costs ~35us for 34KB); each core
sums the 8 slots and runs the small replicated MLP head.
"""

import sys

if "/opt/trn_rl_repo" not in sys.path:
    sys.path.insert(0, "/opt/trn_rl_repo")

import numpy as np
import ml_dtypes

import concourse.bacc as bacc
import concourse.bass as bass
import concourse.mybir as mybir
import concourse.tile as tile
from concourse import masks
from concourse.bass_utils import run_bass_kernel_spmd

NCORES = 8
N = 262144
F = 64
D = 64
DF = 32
H1 = 128
H2 = 64
NSH = N // NCORES          # nodes per core = 32768
NBLK = NSH // 128          # 128-node matmul blocks per core = 256
NSUP = 8                   # DMA super-tiles
BPS = NBLK // NSUP         # blocks per super-tile = 32
MPW = NSH * H1 // 128      # mpnn pack free width = 32768
STW = NSH * F // 128       # state pack free width = 16384
MKW = NSH * D // 128       # mask pack free width = 16384
EPS = 1e-6
SLOPE = 0.1
PK = 66                    # pack columns: 64 dse.T + ssum + ssq

f32 = mybir.dt.float32
bf16 = mybir.dt.bfloat16
f8e4 = mybir.dt.float8e4
u8 = mybir.dt.uint8
ADD = mybir.AluOpType.add
MUL = mybir.AluOpType.mult
SUB = mybir.AluOpType.subtract
AX = mybir.AxisListType.X
IDENT = mybir.ActivationFunctionType.Identity
SQUARE = mybir.ActivationFunctionType.Square
SQRT = mybir.ActivationFunctionType.Sqrt

# cpack free-column layout (f32 columns in one packed const tensor)
C_DFST = 0      # [64, 64]   device_feat_state.T (padded 32->64 rows)
C_W1T = 64      # [64, 128]  W1.T padded
C_W2T = 192     # [64, 128]  W2.T
C_W3T = 320     # [128, 256] W3.T, 4 chunks of [128, 64]
C_B1 = 576      # [128, 1]
C_B2 = 577      # [128, 1]
C_B3 = 578      # [64, 1]
C_W4T = 579     # [64, 1]
C_B4 = 580      # [64, 1]
C_SPRED = 581   # [64, 1]
C_MPRED = 582   # [128, 1]
C_FOLD = 583    # [128, 64]  vertical-stacked I64: folds p and p+64
CW = 648


def build_program(stage="full"):
    nc = bacc.Bacc(
        "TRN2",
        target_bir_lowering=False,
        debug=False,
        enable_asserts=False,
        num_devices=NCORES,
        num_swdge_queues=4,
    )

    x_mpnn = nc.dram_tensor("x_mpnn", [128, MPW], bf16, kind="ExternalInput")
    x_state = nc.dram_tensor("x_state", [128, STW], bf16, kind="ExternalInput")
    x_maskT = nc.dram_tensor("x_maskT", [128, MKW], u8, kind="ExternalInput")
    x_cpack = nc.dram_tensor("x_cpack", [128, CW], f32, kind="ExternalInput")
    y_out = nc.dram_tensor("y_out", [D], f32, kind="ExternalOutput")

    hooks = {}
    with tile.TileContext(nc) as tc:
        emit(nc, tc, x_mpnn, x_state, x_maskT, x_cpack, y_out, hooks,
             stage=stage)

    # The arrival gate (rsem >= 14, incremented only by the 7 peers' remote
    # DMAs) is attached AFTER tile scheduling: the scheduling-pass CoreSim
    # has no peers, so an emit-time wait would deadlock it.  The gate is a
    # standalone DVE EventSemaphore (2 wait slots) emitted with a trivially
    # true rsem>=0 wait to pin its stream position; the real >=14 wait is
    # added here, before codegen.
    if stage == "full":
        import bass_rust
        gate_ins = hooks["gate"].ins
        bass_rust.wait_op(gate_ins, hooks["rsem"], 14, "sem-ge", True)

    nc.compile()

    if stage == "full":
        # Safety: the gate must precede the first remote-slot read (the
        # slot-sum add) in the final DVE stream, else the exchange races.
        seen_gate = False
        for f in nc.m.functions:
            for bb in f.blocks:
                for i in bb.instructions:
                    if i.name == gate_ins.name:
                        seen_gate = True
                    if i.name == hooks["first_read"].ins.name:
                        assert seen_gate, "gate scheduled after slot sums"
        assert seen_gate, "arrival gate missing from compiled program"
    return nc


def emit(nc, tc, x_mpnn, x_state, x_maskT, x_cpack, y_out, hooks,
         stage="full"):
    ctx_pools = []

    def pool(name, bufs, space="SBUF"):
        p = tc.tile_pool(name=name, bufs=bufs, space=space)
        ctx_pools.append(p)
        return p.__enter__()

    cpool = pool("const", 1)
    mpnn_pool = pool("mpnn", 5)
    mask_pool = pool("maskp", 4)
    state_pool = pool("state", 4)
    scr_pool = pool("scr", 2)
    ep_pool = pool("ep", 1)
    recv_pool = pool("recv", 1)
    dram_pool = pool("dram", 1, space="DRAM")
    dse_psum = pool("dsepsum", 1, space="PSUM")
    head_psum = pool("headpsum", 4, space="PSUM")

    # ---- p2p exchange setup: clear sems, pre-generate RDMA descriptors ----
    recv = recv_pool.tile([128, 8 * PK], f32, name="recv", tag="recv")
    pack = recv[:, 0:PK]  # slot 0 doubles as this core's own pack
    if stage == "full":
        # Dummy 4-byte CC AllReduce, triggered first: a collectives-NEFF
        # gets a coordinated multi-core launch (the CC init barrier), which
        # keeps profiled per-core exec spans honest.  Without it the cores
        # launch ms-staggered under NTFF profiling and every span inflates
        # by the stagger (the p2p gate waits for the slowest core).
        cc_in = dram_pool.tile([1, 1], f32, name="cc_in", tag="cc_in")
        cc_out = dram_pool.tile([1, 1], f32, name="cc_out", tag="cc_out",
                                addr_space="Shared")
        zz = ep_pool.tile([1, 1], f32, name="zz", tag="zz")
        nc.vector.memset(zz[:, :], 0.0)
        nc.sync.dma_start(cc_in[:, :], zz[:, :])
        nc.gpsimd.collective_compute(
            "AllReduce",
            ADD,
            replica_groups=[list(range(NCORES))],
            ins=[cc_in[:, :].opt()],
            outs=[cc_out[:, :].opt()],
        )
        rsem = nc.alloc_semaphore("ar_rsem")
        lsem = nc.alloc_semaphore("ar_lsem")
        for k in range(1, 8):
            rdests = [None] * 8
            rdests[k] = (0, k)  # Δrid=0, Δtpb=k: peer tpb = my_tpb ^ k
            nc.gpsimd.remote_dma_broadcast(
                recv[:, k * PK:(k + 1) * PK],
                pack,
                rsem,
                lsem,
                rdests=rdests,
                queue_num=1,
            )

    # ---- constants: one packed DMA ----
    cpk = cpool.tile([128, CW], f32, name="cpk")
    nc.scalar.dma_start(cpk[:, :], x_cpack[:, :])
    ident = cpool.tile([64, 64], f32, name="ident")
    masks.make_identity(nc, ident[:, :])
    zeros = cpool.tile([128, D], f32, name="zeros")
    nc.vector.memset(zeros[:, :], 0.0)


    def leaky(dst, alo_pool=None):
        # in-place leaky relu via mult+max (dst: [p, D] f32 tile ap)
        a = ep_pool.tile([dst.partition_size(), D], f32, name="lk", tag="lk")
        nc.vector.tensor_scalar_mul(a[:, :], dst, SLOPE)
        nc.vector.tensor_max(dst, dst, a[:, :])

    # ---- head precompute (independent of node loop / collective) ----
    # device_feat_embedding.T = leaky(W1 @ norm(dfs).T + b1)  [128, 64]
    dfsT = cpk[0:64, C_DFST:C_DFST + 64]
    mean_f = ep_pool.tile([64, 1], f32, name="mean_f", tag="mean_f")
    nc.vector.tensor_reduce(mean_f[:, :], dfsT, axis=AX, op=ADD)
    nc.vector.tensor_scalar_mul(mean_f[:, :], mean_f[:, :], 1.0 / D)
    sqf = ep_pool.tile([64, D], f32, name="sqf", tag="sqf")
    qf = ep_pool.tile([64, 1], f32, name="qf", tag="qf")
    nc.scalar.activation(sqf[:, :], dfsT, SQUARE)
    nc.vector.tensor_reduce(qf[:, :], sqf[:, :], axis=AX, op=ADD)
    nc.vector.tensor_scalar_mul(qf[:, :], qf[:, :], 1.0 / D)
    varf = ep_pool.tile([64, 1], f32, name="varf", tag="varf")
    nc.vector.tensor_mul(varf[:, :], mean_f[:, :], mean_f[:, :])
    nc.vector.tensor_sub(varf[:, :], qf[:, :], varf[:, :])
    stdf = ep_pool.tile([64, 1], f32, name="stdf", tag="stdf")
    nc.scalar.activation(stdf[:, :], varf[:, :], SQRT)
    nc.vector.tensor_scalar_add(stdf[:, :], stdf[:, :], EPS)
    invf = ep_pool.tile([64, 1], f32, name="invf", tag="invf")
    nc.vector.reciprocal(invf[:, :], stdf[:, :])
    dfsn = ep_pool.tile([64, D], f32, name="dfsn", tag="dfsn")
    nc.vector.tensor_scalar(dfsn[:, :], dfsT, mean_f[:, :], invf[:, :],
                            op0=SUB, op1=MUL)
    psum_dfe = head_psum.tile([H1, D], f32, name="psum_dfe", tag="hp")
    nc.tensor.matmul(psum_dfe[:, :], lhsT=cpk[0:64, C_W1T:C_W1T + H1],
                     rhs=dfsn[:, :], start=True, stop=True)
    dfeT = ep_pool.tile([H1, D], f32, name="dfeT", tag="dfeT")
    nc.scalar.activation(dfeT[:, :], psum_dfe[:, :], IDENT,
                         bias=cpk[:, C_B1:C_B1 + 1])
    leaky(dfeT[:, :])

    # repe = mpnn[pred] broadcast along free axis  [128, 64]
    repe = ep_pool.tile([H1, D], f32, name="repe", tag="repe")
    nc.scalar.activation(repe[:, :], zeros[:, :], IDENT,
                         bias=cpk[:, C_MPRED:C_MPRED + 1])

    # ---- main loop: dse matmuls + state column stats ----
    s_cols = ep_pool.tile([128, NSUP], f32, name="s_cols", tag="s_cols")
    q_cols = ep_pool.tile([128, NSUP], f32, name="q_cols", tag="q_cols")
    psum_dse = dse_psum.tile([D, H1], f32, name="psum_dse", tag="psum_dse")

    # Per-queue sustained rates measured: SWDGE-q0 ~178 B/ns, ACT ~90,
    # SP ~19 (the sync engine's sequencer is clogged with tile sync work,
    # starving its queue -- skip it).  66/34 split over Pool/ACT:
    #   Pool <- mask + state + mpnn cols 0:1664   (1206KB per super-tile)
    #   ACT  <- mpnn cols 1664:4096               (623KB)
    SW = STW // NSUP
    MPC = BPS * H1          # mpnn cols per super-tile = 4096
    MA = 13 * 128
    for i in range(NSUP):
        mk = mask_pool.tile([128, BPS * D], u8, name="mk", tag="mk")
        nc.gpsimd.dma_start(mk[:, :],
                            x_maskT[:, i * BPS * D:(i + 1) * BPS * D])
        mp = mpnn_pool.tile([128, MPC], bf16, name="mp", tag="mp")
        nc.gpsimd.dma_start(mp[:, 0:MA],
                            x_mpnn[:, i * MPC:i * MPC + MA])
        nc.scalar.dma_start(mp[:, MA:MPC],
                            x_mpnn[:, i * MPC + MA:(i + 1) * MPC])
        st = state_pool.tile([128, SW], bf16, name="st", tag="st")
        nc.gpsimd.dma_start(st[:, :],
                            x_state[:, i * SW:(i + 1) * SW])

        if stage != "dmaonly":
            for j in range(BPS):
                b = i * BPS + j
                nc.tensor.matmul(
                    psum_dse[:, :],
                    lhsT=mk[:, j * D:(j + 1) * D].bitcast(f8e4),
                    rhs=mp[:, j * H1:(j + 1) * H1],
                    start=(b == 0),
                    stop=(b == NBLK - 1),
                )

        nc.vector.tensor_reduce(s_cols[:, i:i + 1], st[:, :], axis=AX, op=ADD)
        scr = scr_pool.tile([128, STW // NSUP], bf16, name="scr", tag="scr")
        nc.scalar.activation(scr[:, :], st[:, :], SQUARE,
                             accum_out=q_cols[:, i:i + 1])

    if stage == "dmaonly":
        o = ep_pool.tile([D, 1], f32, name="odbg", tag="odbg")
        nc.vector.tensor_copy(o[:, :], s_cols[0:D, 0:1])
        nc.sync.dma_start(y_out[:], o[:, 0])
        for p in reversed(ctx_pools):
            p.__exit__(None, None, None)
        return

    # ---- fold per-tile stats, transpose dse, build pack ----
    acc_s = ep_pool.tile([128, 1], f32, name="acc_s", tag="acc_s")
    nc.vector.tensor_reduce(acc_s[:, :], s_cols[:, :], axis=AX, op=ADD)
    acc_q = ep_pool.tile([128, 1], f32, name="acc_q", tag="acc_q")
    nc.vector.tensor_reduce(acc_q[:, :], q_cols[:, :], axis=AX, op=ADD)

    dse_sb = ep_pool.tile([D, H1], f32, name="dse_sb", tag="dse_sb")
    nc.vector.tensor_copy(dse_sb[:, :], psum_dse[:, :])
    psum_T = head_psum.tile([H1, D], f32, name="psum_T", tag="hp")
    nc.tensor.transpose(psum_T[:, :], dse_sb[:, :], ident[:, :])

    pack_copies = [
        nc.vector.tensor_copy(pack[:, 0:64], psum_T[:, :]),
        nc.vector.tensor_copy(pack[:, 64:65], acc_s[:, :]),
        nc.vector.tensor_copy(pack[:, 65:66], acc_q[:, :]),
    ]

    if stage == "loop":
        nc.sync.dma_start(y_out[:], pack[0:D, 0].partition_broadcast(D) if False else pack[0, 0:64])
        for p in reversed(ctx_pools):
            p.__exit__(None, None, None)
        return

    # ---- fire the 7 pre-generated sends; sum the 8 slots on arrival ----
    # signals_writable=[pack] makes the trigger WAW-depend on the pack
    # copies above (the preps were emitted before the pack existed, so the
    # deferred source-read alone does not order the trigger after them).
    if stage == "full":
        trig = nc.gpsimd.trigger_dma(count=None, queue_num=1,
                                     signals_writable=[pack])

    # The gate is pinned between the pack copies and the slot sums with
    # explicit scheduler dependency edges -- it has no data operands, so
    # without them tile is free to float it anywhere in the DVE stream.
    from concourse.tile import add_dep_helper

    if stage == "full":
        gate = nc.vector.wait_ge(rsem, 0)
        for pc in pack_copies:
            add_dep_helper(gate.ins, pc.ins, sync=False,
                           reason="arrival gate after pack")
        hooks["gate"] = gate
        hooks["rsem"] = rsem
        red2 = ep_pool.tile([128, 4 * PK], f32, name="red2", tag="red2")
        add1 = nc.vector.tensor_add(red2[:, :], recv[:, 0:4 * PK],
                                    recv[:, 4 * PK:8 * PK])
        add_dep_helper(add1.ins, gate.ins, sync=False,
                       reason="slot sums after arrival gate")
        hooks["first_read"] = add1
        red1 = ep_pool.tile([128, 2 * PK], f32, name="red1", tag="red1")
        nc.vector.tensor_add(red1[:, :], red2[:, 0:2 * PK],
                             red2[:, 2 * PK:4 * PK])
        red = ep_pool.tile([128, PK], f32, name="red", tag="red")
        nc.vector.tensor_add(red[:, :], red1[:, 0:PK], red1[:, PK:2 * PK])
        # end-of-run reset: sems are zero at NEFF load, never cleared at
        # start (a fast peer's sends could arrive before a slow peer's
        # start-clear and be wiped).  The gate passing implies rsem==14 --
        # every arrival of this run is in -- and the next run's arrivals
        # are >= one loop-time away, so a post-gate clear cannot race.
        cl = nc.gpsimd.sem_clear(rsem)
        add_dep_helper(cl.ins, trig.ins, sync=False, reason="clear after trig")
        add_dep_helper(cl.ins, gate.ins, sync=True, reason="clear after gate")
    else:
        # nocomm debug: use the local pack as "red" (numerically wrong by
        # a factor ~8 on the sums, but exercises everything but the p2p).
        red = ep_pool.tile([128, PK], f32, name="red", tag="red")
        nc.vector.tensor_scalar_mul(red[:, :], recv[:, 0:PK], float(NCORES))

    # ---- replicated MLP head ----
    dseT = red[:, 0:64]          # [128 h1, 64 d] global masked sums

    # state per-feature mean / inv(std+eps): fold halves via PE, then stats
    psum_f = head_psum.tile([64, 2], f32, name="psum_f", tag="hp")
    nc.tensor.matmul(psum_f[:, :], lhsT=cpk[:, C_FOLD:C_FOLD + 64],
                     rhs=red[:, 64:66], start=True, stop=True)
    mean_s = ep_pool.tile([64, 1], f32, name="mean_s", tag="mean_s")
    nc.scalar.activation(mean_s[:, :], psum_f[:, 0:1], IDENT, scale=1.0 / N)
    ex2_s = ep_pool.tile([64, 1], f32, name="ex2_s", tag="ex2_s")
    nc.scalar.activation(ex2_s[:, :], psum_f[:, 1:2], IDENT, scale=1.0 / N)
    var_s = ep_pool.tile([64, 1], f32, name="var_s", tag="var_s")
    nc.vector.tensor_mul(var_s[:, :], mean_s[:, :], mean_s[:, :])
    nc.vector.tensor_sub(var_s[:, :], ex2_s[:, :], var_s[:, :])
    std_s = ep_pool.tile([64, 1], f32, name="std_s", tag="std_s")
    nc.scalar.activation(std_s[:, :], var_s[:, :], SQRT)
    nc.vector.tensor_scalar_add(std_s[:, :], std_s[:, :], EPS)
    inv_s = ep_pool.tile([64, 1], f32, name="inv_s", tag="inv_s")
    nc.vector.reciprocal(inv_s[:, :], std_s[:, :])

    # rep_latent.T = leaky(W2 @ norm(state[pred]) + b2) for all D columns
    xn = ep_pool.tile([F, 1], f32, name="xn", tag="xn")
    nc.vector.tensor_scalar(xn[:, :], cpk[0:64, C_SPRED:C_SPRED + 1],
                            mean_s[:, :], inv_s[:, :], op0=SUB, op1=MUL)
    xn_b = ep_pool.tile([F, D], f32, name="xn_b", tag="xn_b")
    nc.scalar.activation(xn_b[:, :], zeros[0:F, :], IDENT, bias=xn[:, :])
    psum_repl = head_psum.tile([H1, D], f32, name="psum_repl", tag="hp")
    nc.tensor.matmul(psum_repl[:, :], lhsT=cpk[0:64, C_W2T:C_W2T + H1],
                     rhs=xn_b[:, :], start=True, stop=True)
    repl = ep_pool.tile([H1, D], f32, name="repl", tag="repl")
    nc.scalar.activation(repl[:, :], psum_repl[:, :], IDENT,
                         bias=cpk[:, C_B2:C_B2 + 1])
    leaky(repl[:, :])

    # dse normalization over the device (free) axis
    mean_d = ep_pool.tile([H1, 1], f32, name="mean_d", tag="mean_d")
    nc.vector.tensor_reduce(mean_d[:, :], dseT, axis=AX, op=ADD)
    nc.vector.tensor_scalar_mul(mean_d[:, :], mean_d[:, :], 1.0 / D)
    sqd = ep_pool.tile([H1, D], f32, name="sqd", tag="sqd")
    qd = ep_pool.tile([H1, 1], f32, name="qd", tag="qd")
    nc.scalar.activation(sqd[:, :], dseT, SQUARE)
    nc.vector.tensor_reduce(qd[:, :], sqd[:, :], axis=AX, op=ADD)
    nc.vector.tensor_scalar_mul(qd[:, :], qd[:, :], 1.0 / D)
    vard = ep_pool.tile([H1, 1], f32, name="vard", tag="vard")
    nc.vector.tensor_mul(vard[:, :], mean_d[:, :], mean_d[:, :])
    nc.vector.tensor_sub(vard[:, :], qd[:, :], vard[:, :])
    stdd = ep_pool.tile([H1, 1], f32, name="stdd", tag="stdd")
    nc.scalar.activation(stdd[:, :], vard[:, :], SQRT)
    nc.vector.tensor_scalar_add(stdd[:, :], stdd[:, :], EPS)
    invd = ep_pool.tile([H1, 1], f32, name="invd", tag="invd")
    nc.vector.reciprocal(invd[:, :], stdd[:, :])
    dsen = ep_pool.tile([H1, D], f32, name="dsen", tag="dsen")
    nc.vector.tensor_scalar(dsen[:, :], dseT, mean_d[:, :], invd[:, :],
                            op0=SUB, op1=MUL)

    # h.T = leaky(W3 @ concat.T + b3): 4 accumulated chunks
    psum_h = head_psum.tile([H2, D], f32, name="psum_h", tag="hp")
    chunks = [dfeT[:, :], repl[:, :], repe[:, :], dsen[:, :]]
    for k in range(4):
        nc.tensor.matmul(psum_h[:, :],
                         lhsT=cpk[:, C_W3T + k * H2:C_W3T + (k + 1) * H2],
                         rhs=chunks[k], start=(k == 0), stop=(k == 3))
    hT = ep_pool.tile([H2, D], f32, name="hT", tag="hT")
    nc.scalar.activation(hT[:, :], psum_h[:, :], IDENT,
                         bias=cpk[0:64, C_B3:C_B3 + 1])
    leaky(hT[:, :])

    # output[d] = sum_j hT[j, d] * W4[0, j] + b4
    psum_o = head_psum.tile([D, 1], f32, name="psum_o", tag="hp")
    nc.tensor.matmul(psum_o[:, :], lhsT=hT[:, :],
                     rhs=cpk[0:64, C_W4T:C_W4T + 1], start=True, stop=True)
    out_sb = ep_pool.tile([D, 1], f32, name="out_sb", tag="out_sb")
    nc.scalar.activation(out_sb[:, :], psum_o[:, :], IDENT,
                         bias=cpk[0:64, C_B4:C_B4 + 1])
    nc.sync.dma_start(y_out[:], out_sb[:, 0])

    for p in reversed(ctx_pools):
        p.__exit__(None, None, None)


_compiled = None


def _get_compiled():
    global _compiled
    if _compiled is None:
        _compiled = build_program()
    return _compiled


def make_in_maps(inputs):
    state = np.ascontiguousarray(np.asarray(inputs["state"], dtype=np.float32))
    dfs = np.asarray(inputs["device_feat_state"], dtype=np.float32)
    mpnn = np.ascontiguousarray(
        np.asarray(inputs["mpnn_forward"], dtype=np.float32))
    W1 = np.asarray(inputs["W1"], dtype=np.float32)
    b1 = np.asarray(inputs["b1"], dtype=np.float32)
    W2 = np.asarray(inputs["W2"], dtype=np.float32)
    b2 = np.asarray(inputs["b2"], dtype=np.float32)
    W3 = np.asarray(inputs["W3"], dtype=np.float32)
    b3 = np.asarray(inputs["b3"], dtype=np.float32)
    W4 = np.asarray(inputs["W4"], dtype=np.float32)
    b4 = np.asarray(inputs["b4"], dtype=np.float32)
    mask = np.asarray(inputs["device_assign_state"])
    assert mask.dtype == np.int32
    pred = int(np.asarray(inputs["pred_node"]))

    cpack = np.zeros((128, CW), np.float32)
    cpack[0:64, C_DFST:C_DFST + 64] = np.pad(dfs.T, ((0, 64 - DF), (0, 0)))
    cpack[0:64, C_W1T:C_W1T + H1] = np.pad(W1.T, ((0, 64 - DF), (0, 0)))
    cpack[0:64, C_W2T:C_W2T + H1] = W2.T
    cpack[:, C_W3T:C_W3T + 256] = (
        W3.T.reshape(4, H1, H2).transpose(1, 0, 2).reshape(H1, 4 * H2))
    cpack[:, C_B1] = b1
    cpack[:, C_B2] = b2
    cpack[0:64, C_B3] = b3
    cpack[0:64, C_W4T] = W4[0]
    cpack[0:64, C_B4] = b4[0]
    cpack[0:64, C_SPRED] = state[pred]
    cpack[:, C_MPRED] = mpnn[pred]
    cpack[:, C_FOLD:C_FOLD + 64] = np.tile(np.eye(64, dtype=np.float32),
                                           (2, 1))

    in_maps = []
    for c in range(NCORES):
        sl = slice(c * NSH, (c + 1) * NSH)
        mp = mpnn[sl]  # [NSH, H1]
        mp_p = np.ascontiguousarray(
            mp.reshape(NBLK, 128, H1).transpose(1, 0, 2).reshape(128, MPW)
        ).astype(ml_dtypes.bfloat16)
        st = state[sl]  # [NSH, F]
        st_p = np.ascontiguousarray(
            np.concatenate([st[:NSH // 2].T, st[NSH // 2:].T], axis=0)
        ).astype(ml_dtypes.bfloat16)
        mk = mask[:, sl]  # [D, NSH] int 0/1
        mk_p = (
            mk.T.reshape(NBLK, 128, D).transpose(1, 0, 2).reshape(128, MKW)
            .astype(np.uint8) * np.uint8(0x38)  # fp8 e4m3 1.0
        )
        in_maps.append({
            "x_mpnn": mp_p,
            "x_state": st_p,
            "x_maskT": np.ascontiguousarray(mk_p),
            "x_cpack": cpack,
        })
    return in_maps


def kernel(**inputs) -> np.ndarray:
    nc = _get_compiled()
    in_maps = make_in_maps(inputs)
    res = run_bass_kernel_spmd(nc, in_maps, core_ids=list(range(NCORES)))
    return np.asarray(res.results[0]["y_out"], dtype=np.float32)


# revision 27
# speedup vs baseline: 1.2084x; 1.1400x over previous
"""Trainium2 Bass kernel for nn_Device_Policy (segment_reduce).

Strategy: shard the node axis N across 8 NeuronCores.  Host-side the
inputs are repacked into DMA/PE-friendly layouts and compressed:

  - mpnn_forward -> bf16, packed [128, 32768] so partition p holds node
    b*128+p of the shard for block b (free offset b*128+h).
  - state -> bf16, packed [128, 16384]: features on partitions, the two
    node-halves of the shard stacked (partition f = feat f of nodes
    0..16383, partition 64+f = feat f of nodes 16384..32767).  Column
    sums / sums-of-squares then reduce along the free axis.
  - mask -> fp8 e4m3 bytes (0x00/0x38), pre-transposed and packed
    [128, 16384] with partition p = node b*128+p, free b*64+d.

Per core the masked segment sum dse[d, h1] accumulates in one PSUM bank
over 256 matmuls (lhsT = fp8 mask block [128n, 64d] stationary, rhs =
bf16 mpnn block [128n, 128h] moving).  The [128, 66] pack (dse.T +
state sums) is exchanged via raw peer-to-peer SWDGE remote DMA (XOR
slotting: core c sends its pack to peer c^k's slot k, k=1..7) instead
of the CC-library AllReduce (which costs ~35us for 34KB); each core
sums the 8 slots and runs the small replicated MLP head.
"""

import sys

if "/opt/trn_rl_repo" not in sys.path:
    sys.path.insert(0, "/opt/trn_rl_repo")

import numpy as np
import ml_dtypes

import concourse.bacc as bacc
import concourse.bass as bass
import concourse.mybir as mybir
import concourse.tile as tile
from concourse import masks
from concourse.bass_utils import run_bass_kernel_spmd

NCORES = 8
N = 262144
F = 64
D = 64
DF = 32
H1 = 128
H2 = 64
NSH = N // NCORES          # nodes per core = 32768
NBLK = NSH // 128          # 128-node matmul blocks per core = 256
NSUP = 8                   # DMA super-tiles
BPS = NBLK // NSUP         # blocks per super-tile = 32
MPW = NSH * H1 // 128      # mpnn pack free width = 32768
STW = NSH * F // 128       # state pack free width = 16384
MKW = NSH * D // 128       # mask pack free width = 16384
EPS = 1e-6
SLOPE = 0.1
PK = 66                    # pack columns: 64 dse.T + ssum + ssq

f32 = mybir.dt.float32
bf16 = mybir.dt.bfloat16
f8e4 = mybir.dt.float8e4
u8 = mybir.dt.uint8
ADD = mybir.AluOpType.add
MUL = mybir.AluOpType.mult
SUB = mybir.AluOpType.subtract
AX = mybir.AxisListType.X
IDENT = mybir.ActivationFunctionType.Identity
SQUARE = mybir.ActivationFunctionType.Square
SQRT = mybir.ActivationFunctionType.Sqrt

# cpack free-column layout (f32 columns in one packed const tensor)
C_DFST = 0      # [64, 64]   device_feat_state.T (padded 32->64 rows)
C_W1T = 64      # [64, 128]  W1.T padded
C_W2T = 192     # [64, 128]  W2.T
C_W3T = 320     # [128, 256] W3.T, 4 chunks of [128, 64]
C_B1 = 576      # [128, 1]
C_B2 = 577      # [128, 1]
C_B3 = 578      # [64, 1]
C_W4T = 579     # [64, 1]
C_B4 = 580      # [64, 1]
C_SPRED = 581   # [64, 1]
C_MPRED = 582   # [128, 1]
C_FOLD = 583    # [128, 64]  vertical-stacked I64: folds p and p+64
CW = 648


def build_program(stage="full"):
    nc = bacc.Bacc(
        "TRN2",
        target_bir_lowering=False,
        debug=False,
        enable_asserts=False,
        num_devices=NCORES,
        num_swdge_queues=4,
    )

    x_mpnn = nc.dram_tensor("x_mpnn", [128, MPW], bf16, kind="ExternalInput")
    x_state = nc.dram_tensor("x_state", [128, STW], bf16, kind="ExternalInput")
    x_maskT = nc.dram_tensor("x_maskT", [128, MKW], u8, kind="ExternalInput")
    x_cpack = nc.dram_tensor("x_cpack", [128, CW], f32, kind="ExternalInput")
    y_out = nc.dram_tensor("y_out", [D], f32, kind="ExternalOutput")

    hooks = {}
    with tile.TileContext(nc) as tc:
        emit(nc, tc, x_mpnn, x_state, x_maskT, x_cpack, y_out, hooks,
             stage=stage)

    # The arrival gate (rsem >= 14, incremented only by the 7 peers' remote
    # DMAs) is attached AFTER tile scheduling: the scheduling-pass CoreSim
    # has no peers, so an emit-time wait would deadlock it.  The gate is a
    # standalone DVE EventSemaphore (2 wait slots) emitted with a trivially
    # true rsem>=0 wait to pin its stream position; the real >=14 wait is
    # added here, before codegen.
    if stage == "full":
        import bass_rust
        gate_ins = hooks["gate"].ins
        bass_rust.wait_op(gate_ins, hooks["rsem"], 14, "sem-ge", True)

    nc.compile()

    if stage == "full":
        # Safety: the gate must precede the first remote-slot read (the
        # slot-sum add) in the final DVE stream, else the exchange races.
        seen_gate = False
        for f in nc.m.functions:
            for bb in f.blocks:
                for i in bb.instructions:
                    if i.name == gate_ins.name:
                        seen_gate = True
                    if i.name == hooks["first_read"].ins.name:
                        assert seen_gate, "gate scheduled after slot sums"
        assert seen_gate, "arrival gate missing from compiled program"
    return nc


def emit(nc, tc, x_mpnn, x_state, x_maskT, x_cpack, y_out, hooks,
         stage="full"):
    ctx_pools = []

    def pool(name, bufs, space="SBUF"):
        p = tc.tile_pool(name=name, bufs=bufs, space=space)
        ctx_pools.append(p)
        return p.__enter__()

    cpool = pool("const", 1)
    mpnn_pool = pool("mpnn", 7)
    mask_pool = pool("maskp", 5)
    state_pool = pool("state", 5)
    scr_pool = pool("scr", 2)
    ep_pool = pool("ep", 1)
    recv_pool = pool("recv", 1)
    dram_pool = pool("dram", 1, space="DRAM")
    dse_psum = pool("dsepsum", 1, space="PSUM")
    head_psum = pool("headpsum", 4, space="PSUM")

    # ---- p2p exchange setup: clear sems, pre-generate RDMA descriptors ----
    recv = recv_pool.tile([128, 8 * PK], f32, name="recv", tag="recv")
    pack = recv[:, 0:PK]  # slot 0 doubles as this core's own pack
    if stage == "full":
        # Dummy 4-byte CC AllReduce, triggered first: a collectives-NEFF
        # gets a coordinated multi-core launch (the CC init barrier), which
        # keeps profiled per-core exec spans honest.  Without it the cores
        # launch ms-staggered under NTFF profiling and every span inflates
        # by the stagger (the p2p gate waits for the slowest core).
        cc_in = dram_pool.tile([1, 1], f32, name="cc_in", tag="cc_in")
        cc_out = dram_pool.tile([1, 1], f32, name="cc_out", tag="cc_out",
                                addr_space="Shared")
        zz = ep_pool.tile([1, 1], f32, name="zz", tag="zz")
        nc.vector.memset(zz[:, :], 0.0)
        nc.sync.dma_start(cc_in[:, :], zz[:, :])
        nc.gpsimd.collective_compute(
            "AllReduce",
            ADD,
            replica_groups=[list(range(NCORES))],
            ins=[cc_in[:, :].opt()],
            outs=[cc_out[:, :].opt()],
        )
        rsem = nc.alloc_semaphore("ar_rsem")
        lsem = nc.alloc_semaphore("ar_lsem")
        for k in range(1, 8):
            rdests = [None] * 8
            rdests[k] = (0, k)  # Δrid=0, Δtpb=k: peer tpb = my_tpb ^ k
            nc.gpsimd.remote_dma_broadcast(
                recv[:, k * PK:(k + 1) * PK],
                pack,
                rsem,
                lsem,
                rdests=rdests,
                queue_num=1,
            )

    # ---- constants: one packed DMA ----
    cpk = cpool.tile([128, CW], f32, name="cpk")
    nc.scalar.dma_start(cpk[:, :], x_cpack[:, :])
    ident = cpool.tile([64, 64], f32, name="ident")
    masks.make_identity(nc, ident[:, :])
    zeros = cpool.tile([128, D], f32, name="zeros")
    nc.vector.memset(zeros[:, :], 0.0)


    def leaky(dst, alo_pool=None):
        # in-place leaky relu via mult+max (dst: [p, D] f32 tile ap)
        a = ep_pool.tile([dst.partition_size(), D], f32, name="lk", tag="lk")
        nc.vector.tensor_scalar_mul(a[:, :], dst, SLOPE)
        nc.vector.tensor_max(dst, dst, a[:, :])

    # ---- head precompute (independent of node loop / collective) ----
    # device_feat_embedding.T = leaky(W1 @ norm(dfs).T + b1)  [128, 64]
    dfsT = cpk[0:64, C_DFST:C_DFST + 64]
    mean_f = ep_pool.tile([64, 1], f32, name="mean_f", tag="mean_f")
    nc.vector.tensor_reduce(mean_f[:, :], dfsT, axis=AX, op=ADD)
    nc.vector.tensor_scalar_mul(mean_f[:, :], mean_f[:, :], 1.0 / D)
    sqf = ep_pool.tile([64, D], f32, name="sqf", tag="sqf")
    qf = ep_pool.tile([64, 1], f32, name="qf", tag="qf")
    nc.scalar.activation(sqf[:, :], dfsT, SQUARE)
    nc.vector.tensor_reduce(qf[:, :], sqf[:, :], axis=AX, op=ADD)
    nc.vector.tensor_scalar_mul(qf[:, :], qf[:, :], 1.0 / D)
    varf = ep_pool.tile([64, 1], f32, name="varf", tag="varf")
    nc.vector.tensor_mul(varf[:, :], mean_f[:, :], mean_f[:, :])
    nc.vector.tensor_sub(varf[:, :], qf[:, :], varf[:, :])
    stdf = ep_pool.tile([64, 1], f32, name="stdf", tag="stdf")
    nc.scalar.activation(stdf[:, :], varf[:, :], SQRT)
    nc.vector.tensor_scalar_add(stdf[:, :], stdf[:, :], EPS)
    invf = ep_pool.tile([64, 1], f32, name="invf", tag="invf")
    nc.vector.reciprocal(invf[:, :], stdf[:, :])
    dfsn = ep_pool.tile([64, D], f32, name="dfsn", tag="dfsn")
    nc.vector.tensor_scalar(dfsn[:, :], dfsT, mean_f[:, :], invf[:, :],
                            op0=SUB, op1=MUL)
    psum_dfe = head_psum.tile([H1, D], f32, name="psum_dfe", tag="hp")
    nc.tensor.matmul(psum_dfe[:, :], lhsT=cpk[0:64, C_W1T:C_W1T + H1],
                     rhs=dfsn[:, :], start=True, stop=True)
    dfeT = ep_pool.tile([H1, D], f32, name="dfeT", tag="dfeT")
    nc.scalar.activation(dfeT[:, :], psum_dfe[:, :], IDENT,
                         bias=cpk[:, C_B1:C_B1 + 1])
    leaky(dfeT[:, :])

    # repe = mpnn[pred] broadcast along free axis  [128, 64]
    repe = ep_pool.tile([H1, D], f32, name="repe", tag="repe")
    nc.scalar.activation(repe[:, :], zeros[:, :], IDENT,
                         bias=cpk[:, C_MPRED:C_MPRED + 1])

    # ---- main loop: dse matmuls + state column stats ----
    s_cols = ep_pool.tile([128, NSUP], f32, name="s_cols", tag="s_cols")
    q_cols = ep_pool.tile([128, NSUP], f32, name="q_cols", tag="q_cols")
    psum_dse = dse_psum.tile([D, H1], f32, name="psum_dse", tag="psum_dse")

    # Per-queue sustained rates measured: SWDGE-q0 ~178 B/ns, ACT ~90,
    # SP ~19 (the sync engine's sequencer is clogged with tile sync work,
    # starving its queue -- skip it).  66/34 split over Pool/ACT:
    #   Pool <- mask + state + mpnn cols 0:1664   (1206KB per super-tile)
    #   ACT  <- mpnn cols 1664:4096               (623KB)
    SW = STW // NSUP
    MPC = BPS * H1          # mpnn cols per super-tile = 4096
    MA = 16 * 128           # Pool:ACT ~= 70:30 of the 1835KB per tile
    for i in range(NSUP):
        mk = mask_pool.tile([128, BPS * D], u8, name="mk", tag="mk")
        nc.gpsimd.dma_start(mk[:, :],
                            x_maskT[:, i * BPS * D:(i + 1) * BPS * D])
        mp = mpnn_pool.tile([128, MPC], bf16, name="mp", tag="mp")
        nc.gpsimd.dma_start(mp[:, 0:MA],
                            x_mpnn[:, i * MPC:i * MPC + MA])
        nc.scalar.dma_start(mp[:, MA:MPC],
                            x_mpnn[:, i * MPC + MA:(i + 1) * MPC])
        st = state_pool.tile([128, SW], bf16, name="st", tag="st")
        nc.gpsimd.dma_start(st[:, :],
                            x_state[:, i * SW:(i + 1) * SW])

        if stage != "dmaonly":
            for j in range(BPS):
                b = i * BPS + j
                nc.tensor.matmul(
                    psum_dse[:, :],
                    lhsT=mk[:, j * D:(j + 1) * D].bitcast(f8e4),
                    rhs=mp[:, j * H1:(j + 1) * H1],
                    start=(b == 0),
                    stop=(b == NBLK - 1),
                )

        nc.vector.tensor_reduce(s_cols[:, i:i + 1], st[:, :], axis=AX, op=ADD)
        scr = scr_pool.tile([128, STW // NSUP], bf16, name="scr", tag="scr")
        nc.scalar.activation(scr[:, :], st[:, :], SQUARE,
                             accum_out=q_cols[:, i:i + 1])

    if stage == "dmaonly":
        o = ep_pool.tile([D, 1], f32, name="odbg", tag="odbg")
        nc.vector.tensor_copy(o[:, :], s_cols[0:D, 0:1])
        nc.sync.dma_start(y_out[:], o[:, 0])
        for p in reversed(ctx_pools):
            p.__exit__(None, None, None)
        return

    # ---- fold per-tile stats, transpose dse, build pack ----
    acc_s = ep_pool.tile([128, 1], f32, name="acc_s", tag="acc_s")
    nc.vector.tensor_reduce(acc_s[:, :], s_cols[:, :], axis=AX, op=ADD)
    acc_q = ep_pool.tile([128, 1], f32, name="acc_q", tag="acc_q")
    nc.vector.tensor_reduce(acc_q[:, :], q_cols[:, :], axis=AX, op=ADD)

    dse_sb = ep_pool.tile([D, H1], f32, name="dse_sb", tag="dse_sb")
    nc.vector.tensor_copy(dse_sb[:, :], psum_dse[:, :])
    psum_T = head_psum.tile([H1, D], f32, name="psum_T", tag="hp")
    nc.tensor.transpose(psum_T[:, :], dse_sb[:, :], ident[:, :])

    pack_copies = [
        nc.vector.tensor_copy(pack[:, 0:64], psum_T[:, :]),
        nc.vector.tensor_copy(pack[:, 64:65], acc_s[:, :]),
        nc.vector.tensor_copy(pack[:, 65:66], acc_q[:, :]),
    ]

    if stage == "loop":
        nc.sync.dma_start(y_out[:], pack[0:D, 0].partition_broadcast(D) if False else pack[0, 0:64])
        for p in reversed(ctx_pools):
            p.__exit__(None, None, None)
        return

    # ---- fire the 7 pre-generated sends; sum the 8 slots on arrival ----
    # signals_writable=[pack] makes the trigger WAW-depend on the pack
    # copies above (the preps were emitted before the pack existed, so the
    # deferred source-read alone does not order the trigger after them).
    if stage == "full":
        trig = nc.gpsimd.trigger_dma(count=None, queue_num=1,
                                     signals_writable=[pack])

    # The gate is pinned between the pack copies and the slot sums with
    # explicit scheduler dependency edges -- it has no data operands, so
    # without them tile is free to float it anywhere in the DVE stream.
    from concourse.tile import add_dep_helper

    if stage == "full":
        gate = nc.vector.wait_ge(rsem, 0)
        for pc in pack_copies:
            add_dep_helper(gate.ins, pc.ins, sync=False,
                           reason="arrival gate after pack")
        hooks["gate"] = gate
        hooks["rsem"] = rsem
        red2 = ep_pool.tile([128, 4 * PK], f32, name="red2", tag="red2")
        add1 = nc.vector.tensor_add(red2[:, :], recv[:, 0:4 * PK],
                                    recv[:, 4 * PK:8 * PK])
        add_dep_helper(add1.ins, gate.ins, sync=False,
                       reason="slot sums after arrival gate")
        hooks["first_read"] = add1
        red1 = ep_pool.tile([128, 2 * PK], f32, name="red1", tag="red1")
        nc.vector.tensor_add(red1[:, :], red2[:, 0:2 * PK],
                             red2[:, 2 * PK:4 * PK])
        red = ep_pool.tile([128, PK], f32, name="red", tag="red")
        nc.vector.tensor_add(red[:, :], red1[:, 0:PK], red1[:, PK:2 * PK])
        # end-of-run reset: sems are zero at NEFF load, never cleared at
        # start (a fast peer's sends could arrive before a slow peer's
        # start-clear and be wiped).  The gate passing implies rsem==14 --
        # every arrival of this run is in -- and the next run's arrivals
        # are >= one loop-time away, so a post-gate clear cannot race.
        cl = nc.gpsimd.sem_clear(rsem)
        add_dep_helper(cl.ins, trig.ins, sync=False, reason="clear after trig")
        add_dep_helper(cl.ins, gate.ins, sync=True, reason="clear after gate")
    else:
        # nocomm debug: use the local pack as "red" (numerically wrong by
        # a factor ~8 on the sums, but exercises everything but the p2p).
        red = ep_pool.tile([128, PK], f32, name="red", tag="red")
        nc.vector.tensor_scalar_mul(red[:, :], recv[:, 0:PK], float(NCORES))

    # ---- replicated MLP head ----
    dseT = red[:, 0:64]          # [128 h1, 64 d] global masked sums

    # state per-feature mean / inv(std+eps): fold halves via PE, then stats
    psum_f = head_psum.tile([64, 2], f32, name="psum_f", tag="hp")
    nc.tensor.matmul(psum_f[:, :], lhsT=cpk[:, C_FOLD:C_FOLD + 64],
                     rhs=red[:, 64:66], start=True, stop=True)
    mean_s = ep_pool.tile([64, 1], f32, name="mean_s", tag="mean_s")
    nc.scalar.activation(mean_s[:, :], psum_f[:, 0:1], IDENT, scale=1.0 / N)
    ex2_s = ep_pool.tile([64, 1], f32, name="ex2_s", tag="ex2_s")
    nc.scalar.activation(ex2_s[:, :], psum_f[:, 1:2], IDENT, scale=1.0 / N)
    var_s = ep_pool.tile([64, 1], f32, name="var_s", tag="var_s")
    nc.vector.tensor_mul(var_s[:, :], mean_s[:, :], mean_s[:, :])
    nc.vector.tensor_sub(var_s[:, :], ex2_s[:, :], var_s[:, :])
    std_s = ep_pool.tile([64, 1], f32, name="std_s", tag="std_s")
    nc.scalar.activation(std_s[:, :], var_s[:, :], SQRT)
    nc.vector.tensor_scalar_add(std_s[:, :], std_s[:, :], EPS)
    inv_s = ep_pool.tile([64, 1], f32, name="inv_s", tag="inv_s")
    nc.vector.reciprocal(inv_s[:, :], std_s[:, :])

    # rep_latent.T = leaky(W2 @ norm(state[pred]) + b2) for all D columns
    xn = ep_pool.tile([F, 1], f32, name="xn", tag="xn")
    nc.vector.tensor_scalar(xn[:, :], cpk[0:64, C_SPRED:C_SPRED + 1],
                            mean_s[:, :], inv_s[:, :], op0=SUB, op1=MUL)
    xn_b = ep_pool.tile([F, D], f32, name="xn_b", tag="xn_b")
    nc.scalar.activation(xn_b[:, :], zeros[0:F, :], IDENT, bias=xn[:, :])
    psum_repl = head_psum.tile([H1, D], f32, name="psum_repl", tag="hp")
    nc.tensor.matmul(psum_repl[:, :], lhsT=cpk[0:64, C_W2T:C_W2T + H1],
                     rhs=xn_b[:, :], start=True, stop=True)
    repl = ep_pool.tile([H1, D], f32, name="repl", tag="repl")
    nc.scalar.activation(repl[:, :], psum_repl[:, :], IDENT,
                         bias=cpk[:, C_B2:C_B2 + 1])
    leaky(repl[:, :])

    # dse normalization over the device (free) axis
    mean_d = ep_pool.tile([H1, 1], f32, name="mean_d", tag="mean_d")
    nc.vector.tensor_reduce(mean_d[:, :], dseT, axis=AX, op=ADD)
    nc.vector.tensor_scalar_mul(mean_d[:, :], mean_d[:, :], 1.0 / D)
    sqd = ep_pool.tile([H1, D], f32, name="sqd", tag="sqd")
    qd = ep_pool.tile([H1, 1], f32, name="qd", tag="qd")
    nc.scalar.activation(sqd[:, :], dseT, SQUARE)
    nc.vector.tensor_reduce(qd[:, :], sqd[:, :], axis=AX, op=ADD)
    nc.vector.tensor_scalar_mul(qd[:, :], qd[:, :], 1.0 / D)
    vard = ep_pool.tile([H1, 1], f32, name="vard", tag="vard")
    nc.vector.tensor_mul(vard[:, :], mean_d[:, :], mean_d[:, :])
    nc.vector.tensor_sub(vard[:, :], qd[:, :], vard[:, :])
    stdd = ep_pool.tile([H1, 1], f32, name="stdd", tag="stdd")
    nc.scalar.activation(stdd[:, :], vard[:, :], SQRT)
    nc.vector.tensor_scalar_add(stdd[:, :], stdd[:, :], EPS)
    invd = ep_pool.tile([H1, 1], f32, name="invd", tag="invd")
    nc.vector.reciprocal(invd[:, :], stdd[:, :])
    dsen = ep_pool.tile([H1, D], f32, name="dsen", tag="dsen")
    nc.vector.tensor_scalar(dsen[:, :], dseT, mean_d[:, :], invd[:, :],
                            op0=SUB, op1=MUL)

    # h.T = leaky(W3 @ concat.T + b3): 4 accumulated chunks
    psum_h = head_psum.tile([H2, D], f32, name="psum_h", tag="hp")
    chunks = [dfeT[:, :], repl[:, :], repe[:, :], dsen[:, :]]
    for k in range(4):
        nc.tensor.matmul(psum_h[:, :],
                         lhsT=cpk[:, C_W3T + k * H2:C_W3T + (k + 1) * H2],
                         rhs=chunks[k], start=(k == 0), stop=(k == 3))
    hT = ep_pool.tile([H2, D], f32, name="hT", tag="hT")
    nc.scalar.activation(hT[:, :], psum_h[:, :], IDENT,
                         bias=cpk[0:64, C_B3:C_B3 + 1])
    leaky(hT[:, :])

    # output[d] = sum_j hT[j, d] * W4[0, j] + b4
    psum_o = head_psum.tile([D, 1], f32, name="psum_o", tag="hp")
    nc.tensor.matmul(psum_o[:, :], lhsT=hT[:, :],
                     rhs=cpk[0:64, C_W4T:C_W4T + 1], start=True, stop=True)
    out_sb = ep_pool.tile([D, 1], f32, name="out_sb", tag="out_sb")
    nc.scalar.activation(out_sb[:, :], psum_o[:, :], IDENT,
                         bias=cpk[0:64, C_B4:C_B4 + 1])
    nc.sync.dma_start(y_out[:], out_sb[:, 0])

    for p in reversed(ctx_pools):
        p.__exit__(None, None, None)


_compiled = None


def _get_compiled():
    global _compiled
    if _compiled is None:
        _compiled = build_program()
    return _compiled


def make_in_maps(inputs):
    state = np.ascontiguousarray(np.asarray(inputs["state"], dtype=np.float32))
    dfs = np.asarray(inputs["device_feat_state"], dtype=np.float32)
    mpnn = np.ascontiguousarray(
        np.asarray(inputs["mpnn_forward"], dtype=np.float32))
    W1 = np.asarray(inputs["W1"], dtype=np.float32)
    b1 = np.asarray(inputs["b1"], dtype=np.float32)
    W2 = np.asarray(inputs["W2"], dtype=np.float32)
    b2 = np.asarray(inputs["b2"], dtype=np.float32)
    W3 = np.asarray(inputs["W3"], dtype=np.float32)
    b3 = np.asarray(inputs["b3"], dtype=np.float32)
    W4 = np.asarray(inputs["W4"], dtype=np.float32)
    b4 = np.asarray(inputs["b4"], dtype=np.float32)
    mask = np.asarray(inputs["device_assign_state"])
    assert mask.dtype == np.int32
    pred = int(np.asarray(inputs["pred_node"]))

    cpack = np.zeros((128, CW), np.float32)
    cpack[0:64, C_DFST:C_DFST + 64] = np.pad(dfs.T, ((0, 64 - DF), (0, 0)))
    cpack[0:64, C_W1T:C_W1T + H1] = np.pad(W1.T, ((0, 64 - DF), (0, 0)))
    cpack[0:64, C_W2T:C_W2T + H1] = W2.T
    cpack[:, C_W3T:C_W3T + 256] = (
        W3.T.reshape(4, H1, H2).transpose(1, 0, 2).reshape(H1, 4 * H2))
    cpack[:, C_B1] = b1
    cpack[:, C_B2] = b2
    cpack[0:64, C_B3] = b3
    cpack[0:64, C_W4T] = W4[0]
    cpack[0:64, C_B4] = b4[0]
    cpack[0:64, C_SPRED] = state[pred]
    cpack[:, C_MPRED] = mpnn[pred]
    cpack[:, C_FOLD:C_FOLD + 64] = np.tile(np.eye(64, dtype=np.float32),
                                           (2, 1))

    in_maps = []
    for c in range(NCORES):
        sl = slice(c * NSH, (c + 1) * NSH)
        mp = mpnn[sl]  # [NSH, H1]
        mp_p = np.ascontiguousarray(
            mp.reshape(NBLK, 128, H1).transpose(1, 0, 2).reshape(128, MPW)
        ).astype(ml_dtypes.bfloat16)
        st = state[sl]  # [NSH, F]
        st_p = np.ascontiguousarray(
            np.concatenate([st[:NSH // 2].T, st[NSH // 2:].T], axis=0)
        ).astype(ml_dtypes.bfloat16)
        mk = mask[:, sl]  # [D, NSH] int 0/1
        mk_p = (
            mk.T.reshape(NBLK, 128, D).transpose(1, 0, 2).reshape(128, MKW)
            .astype(np.uint8) * np.uint8(0x38)  # fp8 e4m3 1.0
        )
        in_maps.append({
            "x_mpnn": mp_p,
            "x_state": st_p,
            "x_maskT": np.ascontiguousarray(mk_p),
            "x_cpack": cpack,
        })
    return in_maps


def kernel(**inputs) -> np.ndarray:
    nc = _get_compiled()
    in_maps = make_in_maps(inputs)
    res = run_bass_kernel_spmd(nc, in_maps, core_ids=list(range(NCORES)))
    return np.asarray(res.results[0]["y_out"], dtype=np.float32)
